# revision 11
# baseline (speedup 1.0000x reference)
"""Multi-head attention (B=4, S=2048, E=1024, H=16, D=64) on 8 TRN2 cores.

Sharding: core c handles batch b = c//2, query half = c%2 (1024 queries).
Each core computes K/V over its batch's full sequence (duplicated between the
two half-cores of a batch -- cheaper at these sizes than any collective),
attention for all 16 heads over its 1024 queries, and the output projection
for its output chunk. Outputs are disjoint -> host gather is concatenation.

The host rotates each core's sequence so its query block is always rows
0:1024 (attention is permutation-invariant over keys), pre-transposes the
weights and activations (pure layout prep) so the e-contraction projections
have e on partitions, and converts everything to bf16 (rel-err budget 2e-2;
bf16 lands ~3e-3).

bf16 operands keep the PE at full rate, enable Fast Weight Load, halve DMA
bytes, and remove all dtype-staging copies: DMAs land directly in the SBUF
tiles the matmuls read. PSUM accumulation stays fp32.

Scores matmuls use contraction 64 (head dim) at base partitions 0/64, which
bass auto-lowers to 64x128 row tiles (0,0)/(64,0) -- on hardware the two
head matmuls run concurrently in the two half-arrays.

Schedule: the first head-pair's weight DMAs go out before the x DMAs (so the
PE starts ~5us in, not ~16us); head-pair hp+1's projection instructions are
interleaved into hp's attention stream; the out-projection weights prefetch
mid-attention; the first query-half's out-projection interleaves into the
last head-pair's attention; softmax state is copied out of PSUM immediately
so the po banks recycle without waiting on the normalize chain.
"""

from contextlib import ExitStack

import numpy as np

import concourse.tile as tile
from concourse import bacc, mybir
from concourse.bass_utils import run_bass_kernel_spmd

dt = mybir.dt
AF = mybir.ActivationFunctionType

B, S, E, H, D = 4, 2048, 1024, 16, 64
N_CORES = 8
SQ = 1024          # queries per core
P = 128
EC = E // P        # 8 e-chunks
TC = S // P        # 16 t-chunks (keys)
QC = SQ // P       # 8 query chunks
HP = H // 2        # 8 head-pairs
XCH = 4            # xT token chunks (512 tokens each)


def _emit(nc, tc, xt_d, wqt, wkt, wvt, wot, bo, y):
    f32, bf16 = dt.float32, dt.bfloat16

    with ExitStack() as ctx:
        const = ctx.enter_context(tc.tile_pool(name="const", bufs=1))
        ps_p = ctx.enter_context(
            tc.tile_pool(name="ps_p", bufs=1, space="PSUM"))
        on_pool = ctx.enter_context(tc.tile_pool(name="on", bufs=1))
        wo_pool = ctx.enter_context(tc.tile_pool(name="wo", bufs=2))
        yp = ctx.enter_context(tc.tile_pool(name="yp", bufs=2))

        ones_col = const.tile([P, 1], bf16)
        nc.vector.memset(ones_col[:], 1.0)

        # warm the PE (p-state / HAM) with dependency-free matmuls while the
        # first DMAs are in flight
        wu = const.tile([P, 512], bf16)
        nc.vector.memset(wu[:], 0.0)

        # attention output, [e, q] layout: partition j of tile (qh, hp) is
        # e-row hp*128+j (head 2hp on partitions 0:64, 2hp+1 on 64:128).
        # One tile per (query-half, head-pair) so out-projection reads only
        # depend on the normalizes that actually produced them.
        onT = [[on_pool.tile([P, 512], bf16, tag=f"on{qh}_{hp}",
                             name=f"onT{qh}_{hp}")
                for hp in range(HP)]
               for qh in range(2)]
        wo_rs = []
        bo_rep = [None]

        def prefetch_wo():
            bo_one = wo_pool.tile([1, E], f32, tag="bo1")
            nc.sync.dma_start(bo_one[:], bo[:])
            bo_rep[0] = wo_pool.tile([P, E], f32, tag="bor", name="bo_rep")
            nc.gpsimd.partition_broadcast(bo_rep[0][:], bo_one[:])
            wot_view = wot.rearrange("(o p) f -> p o f", p=P)
            for nf in range(E // 512):
                wo_sb = wo_pool.tile([P, EC, 512], bf16, tag="wosb")
                nc.sync.dma_start(
                    wo_sb[:], wot_view[:, :, nf * 512:(nf + 1) * 512])
                wo_rs.append(wo_sb)

        def outproj_ops(qh, pool):
            """Out-projection for query-half qh: reads onT[qh] tiles."""
            ops = []
            st = {}
            for qc in range(4 * qh, 4 * qh + 4):
                for nf in range(E // 512):
                    def palloc(qc=qc, nf=nf):
                        st["py"] = pool.tile([P, 512], f32, tag="PROJ",
                                             name=f"py{qc}_{nf}")
                    ops.append(palloc)
                    for hp in range(HP):
                        def pmm(hp=hp, qc=qc, nf=nf):
                            nc.tensor.matmul(
                                st["py"][:],
                                onT[qh][hp][:, (qc % 4) * P:(qc % 4 + 1) * P],
                                wo_rs[nf][:, hp, :],
                                start=(hp == 0), stop=(hp == HP - 1))
                        ops.append(pmm)

                    def pout(qc=qc, nf=nf):
                        y_sb = yp.tile([P, 512], f32, tag="ysb")
                        nc.vector.tensor_add(
                            y_sb[:], st["py"][:],
                            bo_rep[0][:, nf * 512:(nf + 1) * 512])
                        nc.sync.dma_start(
                            y[qc * P:(qc + 1) * P, nf * 512:(nf + 1) * 512],
                            y_sb[:])
                    ops.append(pout)
            return ops

        with ExitStack() as actx:
            ps = actx.enter_context(
                tc.tile_pool(name="ps", bufs=2, space="PSUM"))
            wu_p = ps.tile([P, 512], f32, tag="S", name="wu_p")
            for _ in range(16):
                nc.tensor.matmul(wu_p[:], wu[:, 0:P], wu[:],
                                 start=True, stop=True)
            ps_o = actx.enter_context(
                tc.tile_pool(name="ps_o", bufs=3, space="PSUM"))
            xt_pool = actx.enter_context(tc.tile_pool(name="xt", bufs=1))
            w1 = actx.enter_context(tc.tile_pool(name="w1", bufs=1))
            w2 = actx.enter_context(tc.tile_pool(name="w2", bufs=2))
            vp_pool = actx.enter_context(tc.tile_pool(name="vp", bufs=2))
            ut_pool = actx.enter_context(tc.tile_pool(name="ut", bufs=6))

            # xT in SBUF (bf16), 4 token-chunk tiles; queries are chunks 0:2
            xTs = [xt_pool.tile([P, EC, 512], bf16, tag=f"xt{i}",
                            name=f"xT{i}")
                   for i in range(XCH)]
            xt_view = xt_d.rearrange("(o p) t -> p o t", p=P)

            def xtc(ec, t0, t1):
                """View of xT columns t0:t1 (within one 512 chunk) at ec."""
                c = t0 // 512
                assert t1 <= (c + 1) * 512
                return xTs[c][:, ec, t0 - c * 512:t1 - c * 512]

            qt_t, kt_t, vp_t = {}, {}, {}

            def proj_ops(hp, by_chunk=False):
                pre = []
                qb, kb, vb = [], [], []
                st = {}
                # prologue weights ride the Activation hwdge queue so they
                # don't delay the x chunks on the SP queue
                dma_eng = nc.scalar if hp == 0 else nc.sync

                def wload():
                    st["w"] = w1.tile([P, EC, 2, P], bf16, tag="wdma",
                                      name=f"w{hp}")
                    for wi, w_dram in enumerate((wqt, wkt)):
                        dma_eng.dma_start(
                            st["w"][:, :, wi, :],
                            w_dram.rearrange("(o p) f -> p o f", p=P)[
                                :, :, hp * P:(hp + 1) * P])
                pre.append(wload)

                if hp % 2 == 0:
                    def vload():
                        st["wv"] = w1.tile([P, EC, 2 * P], bf16, tag="wdma_v",
                                           name=f"wv{hp}")
                        dma_eng.dma_start(
                            st["wv"][:],
                            wvt.rearrange("(o p) f -> p o f", p=P)[
                                :, :, hp * P:(hp + 2) * P])
                        vp_t[hp // 2] = vp_pool.tile(
                            [P, TC, 4, 65], bf16, tag="vp",
                            name=f"vp{hp // 2}")
                        nc.vector.tensor_copy(
                            vp_t[hp // 2][:, :, :, 64:65],
                            ones_col[:, None, None, :].to_broadcast(
                                [P, TC, 4, 1]))
                    pre.append(vload)

                # QT: two q-halves, each accumulated over ec in own psum
                for nq in range(SQ // 512):
                    blk = []

                    def qalloc(nq=nq):
                        if nq == 0:
                            qt_t[hp] = w2.tile([P, SQ], bf16, tag="qt",
                                               name=f"qt{hp}")
                        st["pq"] = ps_p.tile([P, 512], f32, tag="PROJ",
                                             name=f"pq{hp}_{nq}")
                    blk.append(qalloc)
                    for ec in range(EC):
                        def qmm(ec=ec, nq=nq):
                            nc.tensor.matmul(
                                st["pq"][:], st["w"][:, ec, 0],
                                xtc(ec, nq * 512, (nq + 1) * 512),
                                start=(ec == 0), stop=(ec == EC - 1))
                        blk.append(qmm)

                    def qcopy(nq=nq):
                        nc.vector.tensor_copy(
                            qt_t[hp][:, nq * 512:(nq + 1) * 512], st["pq"][:])
                    blk.append(qcopy)
                    qb.append(blk)

                # KT: four 512-chunks
                for nk in range(S // 512):
                    blk = []

                    def kalloc(nk=nk):
                        if nk == 0:
                            kt_t[hp] = w2.tile([P, S], bf16, tag="kt",
                                               name=f"kt{hp}")
                        st["pk"] = ps_p.tile([P, 512], f32, tag="PROJ",
                                             name=f"pk{hp}_{nk}")
                    blk.append(kalloc)
                    for ec in range(EC):
                        def kmm(ec=ec, nk=nk):
                            nc.tensor.matmul(
                                st["pk"][:], st["w"][:, ec, 1],
                                xtc(ec, nk * 512, (nk + 1) * 512),
                                start=(ec == 0), stop=(ec == EC - 1))
                        blk.append(kmm)

                    def kcopy(nk=nk):
                        nc.vector.tensor_copy(
                            kt_t[hp][:, nk * 512:(nk + 1) * 512], st["pk"][:])
                    blk.append(kcopy)
                    kb.append(blk)

                # V for the pair (hp, hp+1) on even hp: out free dim 256
                if hp % 2 == 0:
                    for tc_i in range(TC):
                        blk = []

                        def valloc(tc_i=tc_i):
                            st["pv"] = ps_p.tile([P, 512], f32, tag="PROJ",
                                                 name=f"pv{hp}_{tc_i}")
                        blk.append(valloc)
                        for ec in range(EC):
                            def vmm(ec=ec, tc_i=tc_i):
                                nc.tensor.matmul(
                                    st["pv"][:, :256],
                                    xtc(ec, tc_i * P, (tc_i + 1) * P),
                                    st["wv"][:, ec, :],
                                    start=(ec == 0), stop=(ec == EC - 1))
                            blk.append(vmm)

                        def vcopy(tc_i=tc_i):
                            nc.vector.tensor_copy(
                                vp_t[hp // 2][:, tc_i, :, 0:64],
                                st["pv"][:, :256].rearrange(
                                    "p (h d) -> p h d", h=4))
                        blk.append(vcopy)
                        vb.append(blk)

                ops = []
                if by_chunk:
                    # group by x token-chunk so hp0's projections start as
                    # each x DMA chunk lands (q/k chunk i and v tc 4i..4i+3
                    # read only x chunk i)
                    for c in range(XCH):
                        if c < len(qb):
                            ops += qb[c]
                        ops += kb[c]
                        for blk in vb[4 * c:4 * c + 4]:
                            ops += blk
                else:
                    for blk in qb + kb + vb:
                        ops += blk
                return pre, ops

            # prologue: head-pair 0's weight DMAs, then x DMAs, then hp0 proj
            pre0, rest0 = proj_ops(0, by_chunk=True)
            for op in pre0:
                op()
            for i in range(XCH):
                nc.sync.dma_start(xTs[i][:], xt_view[:, :, i * 512:(i + 1) * 512])
            for op in rest0:
                op()

            for hp in range(HP):
                qt, kt = qt_t[hp], kt_t[hp]
                vp = vp_t[hp // 2]
                ha, hb = 2 * (hp % 2), 2 * (hp % 2) + 1
                if hp + 1 < HP:
                    pre, rest = proj_ops(hp + 1)
                    allops = pre + rest
                    nxt = {0: allops[:len(allops) // 2],
                           1: allops[len(allops) // 2:]}
                else:
                    # last head-pair: interleave first-half out-projection
                    # into the second query-half's attention stream
                    nxt = {0: [], 1: outproj_ops(0, ps_p)}

                for qh in range(2):
                    ops_q = nxt[qh]
                    n_emit = 0
                    po_a = ps_o.tile([65, 512], f32, tag="po")
                    po_b = ps_o.tile([65, 512], f32, tag="po")
                    qs = slice(qh * 512, (qh + 1) * 512)
                    for kc in range(TC):
                        sc = ps.tile([P, 1024], f32, tag="S")
                        nc.tensor.matmul(
                            sc[:, 0:512], kt[0:64, kc * P:(kc + 1) * P],
                            qt[0:64, qs], start=True, stop=True)
                        nc.tensor.matmul(
                            sc[:, 512:1024], kt[64:128, kc * P:(kc + 1) * P],
                            qt[64:128, qs], start=True, stop=True)
                        ut = ut_pool.tile([P, 1024], bf16, tag="ut")
                        nc.scalar.activation(
                            ut[:], sc[:], AF.Exp, scale=0.125)
                        nc.tensor.matmul(
                            po_a[:], vp[:, kc, ha], ut[:, 0:512],
                            start=(kc == 0), stop=(kc == TC - 1))
                        nc.tensor.matmul(
                            po_b[:], vp[:, kc, hb], ut[:, 512:1024],
                            start=(kc == 0), stop=(kc == TC - 1))
                        # interleave pipelined work
                        target = len(ops_q) * (kc + 1) // TC
                        while n_emit < target:
                            ops_q[n_emit]()
                            n_emit += 1
                    assert n_emit == len(ops_q)

                    # evacuate po immediately (frees the PSUM banks; a DVE
                    # op may read PSUM across partitions, so head b lands at
                    # partitions 64:128), then normalize: row 64 of each po
                    # is the softmax denominator. partition_broadcast only
                    # writes correctly from base 0: broadcast into a full
                    # tile, slice at read time.
                    po_s = w1.tile([P, 512], f32, tag="po_s")
                    nc.vector.tensor_copy(po_s[0:64, :], po_a[0:64, :])
                    rcp_a = w1.tile([1, 512], f32, tag="rcp_a")
                    nc.vector.reciprocal(rcp_a[:], po_a[64:65, :])
                    nc.vector.tensor_copy(po_s[64:128, :], po_b[0:64, :])
                    rcp_b = w1.tile([1, 512], f32, tag="rcp_b")
                    nc.vector.reciprocal(rcp_b[:], po_b[64:65, :])
                    brec_a = w1.tile([P, 512], f32, tag="brec_a")
                    nc.gpsimd.partition_broadcast(brec_a[:], rcp_a[:])
                    brec_b = w1.tile([P, 512], f32, tag="brec_b")
                    nc.gpsimd.partition_broadcast(brec_b[:], rcp_b[:])
                    nc.vector.tensor_mul(
                        onT[qh][hp][0:64, :], po_s[0:64, :], brec_a[0:64, :])
                    nc.vector.tensor_mul(
                        onT[qh][hp][64:128, :], po_s[64:128, :],
                        brec_b[64:128, :])

                # prefetch out-projection weights mid-attention
                if hp == 2:
                    prefetch_wo()

        # ---- output projection tail: second query-half ----
        with ExitStack() as dctx:
            ps_t = dctx.enter_context(
                tc.tile_pool(name="ps_t", bufs=3, space="PSUM"))
            for op in outproj_ops(1, ps_t):
                op()


def _build_kernel(reps=1):
    nc = bacc.Bacc("TRN2", target_bir_lowering=False, debug=False,
                   num_devices=N_CORES)
    xt_d = nc.dram_tensor("xt", [E, S], dt.bfloat16,
                          kind="ExternalInput").ap()
    wqt = nc.dram_tensor("wqt", [E, E], dt.bfloat16,
                         kind="ExternalInput").ap()
    wkt = nc.dram_tensor("wkt", [E, E], dt.bfloat16,
                         kind="ExternalInput").ap()
    wvt = nc.dram_tensor("wvt", [E, E], dt.bfloat16,
                         kind="ExternalInput").ap()
    wot = nc.dram_tensor("wot", [E, E], dt.bfloat16,
                         kind="ExternalInput").ap()
    bo = nc.dram_tensor("bo", [1, E], dt.float32, kind="ExternalInput").ap()
    y = nc.dram_tensor("y", [SQ, E], dt.float32, kind="ExternalOutput").ap()

    with tile.TileContext(nc) as tc:
        for _ in range(reps):
            _emit(nc, tc, xt_d, wqt, wkt, wvt, wot, bo, y)
    nc.compile()
    return nc


_NC_CACHE = None


def _bf16(a):
    import ml_dtypes
    return np.ascontiguousarray(np.asarray(a, np.float32).astype(
        ml_dtypes.bfloat16))


def make_in_maps(x, Wq, Wk, Wv, Wo, bo):
    x = np.asarray(x, np.float32)
    wqt = _bf16(np.asarray(Wq, np.float32).T)
    wkt = _bf16(np.asarray(Wk, np.float32).T)
    wvt = _bf16(np.asarray(Wv, np.float32).T)
    wot = _bf16(np.asarray(Wo, np.float32).T)
    bo_ = np.ascontiguousarray(np.asarray(bo, np.float32).reshape(1, E))

    in_maps = []
    for c in range(N_CORES):
        b, half = c // 2, c % 2
        # rotate so this core's query block is rows 0:SQ (keys are a
        # permutation of the sequence -- attention is invariant to key order)
        xt_rot = _bf16(np.roll(x[b], -half * SQ, axis=0).T)
        in_maps.append({"xt": xt_rot, "wqt": wqt, "wkt": wkt, "wvt": wvt,
                        "wot": wot, "bo": bo_})
    return in_maps


def get_nc(reps=1):
    global _NC_CACHE
    if _NC_CACHE is None:
        _NC_CACHE = {}
    if reps not in _NC_CACHE:
        _NC_CACHE[reps] = _build_kernel(reps)
    return _NC_CACHE[reps]


def kernel(x, Wq, Wk, Wv, Wo, bo):
    nc = get_nc()
    in_maps = make_in_maps(x, Wq, Wk, Wv, Wo, bo)
    res = run_bass_kernel_spmd(nc, in_maps, core_ids=list(range(N_CORES)))
    out = np.empty((B, S, E), np.float32)
    for c in range(N_CORES):
        b, half = c // 2, c % 2
        out[b, half * SQ:(half + 1) * SQ, :] = res.results[c]["y"]
    return out


# revision 22
# speedup vs baseline: 1.0535x; 1.0535x over previous
"""Multi-head attention (B=4, S=2048, E=1024, H=16, D=64) on 8 TRN2 cores.

Sharding: core c handles batch b = c//2, query half = c%2 (1024 queries).
Each core computes K/V over its batch's full sequence (duplicated between the
two half-cores of a batch -- cheaper at these sizes than any collective),
attention for all 16 heads over its 1024 queries, and the output projection
for its output chunk. Outputs are disjoint -> host gather is concatenation.

The host rotates each core's sequence so its query block is always rows
0:1024 (attention is permutation-invariant over keys), pre-transposes the
weights and activations (pure layout prep) so the e-contraction projections
have e on partitions, and converts everything to bf16 (rel-err budget 2e-2;
bf16 lands ~3e-3).

bf16 operands keep the PE at full rate, enable Fast Weight Load, halve DMA
bytes, and remove all dtype-staging copies: DMAs land directly in the SBUF
tiles the matmuls read. PSUM accumulation stays fp32.

Scores matmuls use contraction 64 (head dim) at base partitions 0/64, which
bass auto-lowers to 64x128 row tiles (0,0)/(64,0) -- on hardware the two
head matmuls run concurrently in the two half-arrays.

Schedule: the first head-pair's weight DMAs go out before the x DMAs (so the
PE starts ~5us in, not ~16us); head-pair hp+1's projection instructions are
interleaved into hp's attention stream; the out-projection weights prefetch
mid-attention; the first query-half's out-projection interleaves into the
last head-pair's attention; softmax state is copied out of PSUM immediately
so the po banks recycle without waiting on the normalize chain.
"""

from contextlib import ExitStack

import numpy as np

import concourse.tile as tile
from concourse import bacc, mybir
from concourse.bass_utils import run_bass_kernel_spmd

dt = mybir.dt
AF = mybir.ActivationFunctionType

B, S, E, H, D = 4, 2048, 1024, 16, 64
N_CORES = 8
SQ = 1024          # queries per core
P = 128
EC = E // P        # 8 e-chunks
TC = S // P        # 16 t-chunks (keys)
QC = SQ // P       # 8 query chunks
HP = H // 2        # 8 head-pairs
XCH = 4            # xT token chunks (512 tokens each)


def _emit(nc, tc, xt_d, wqt, wkt, wvt, wot, bo, y):
    f32, bf16 = dt.float32, dt.bfloat16

    with ExitStack() as ctx:
        const = ctx.enter_context(tc.tile_pool(name="const", bufs=1))
        ps_p = ctx.enter_context(
            tc.tile_pool(name="ps_p", bufs=1, space="PSUM"))
        on_pool = ctx.enter_context(tc.tile_pool(name="on", bufs=1))
        wo_pool = ctx.enter_context(tc.tile_pool(name="wo", bufs=2))
        yp = ctx.enter_context(tc.tile_pool(name="yp", bufs=4))

        ones_col = const.tile([P, 1], bf16)
        nc.vector.memset(ones_col[:], 1.0)

        # warm the PE (p-state / HAM) with dependency-free matmuls while the
        # first DMAs are in flight
        wu = const.tile([P, 512], bf16)
        nc.vector.memset(wu[:], 0.0)

        # attention output, [e, q] layout: partition j of tile (qh, hp) is
        # e-row hp*128+j (head 2hp on partitions 0:64, 2hp+1 on 64:128).
        # One tile per (query-half, head-pair) so out-projection reads only
        # depend on the normalizes that actually produced them.
        onT = [[on_pool.tile([P, 512], bf16, tag=f"on{qh}_{hp}",
                             name=f"onT{qh}_{hp}")
                for hp in range(HP)]
               for qh in range(2)]
        wo_rs = []
        bo_rep = [None]

        def prefetch_wo():
            bo_one = wo_pool.tile([1, E], f32, tag="bo1")
            nc.sync.dma_start(bo_one[:], bo[:])
            bo_rep[0] = wo_pool.tile([P, E], f32, tag="bor", name="bo_rep")
            nc.gpsimd.partition_broadcast(bo_rep[0][:], bo_one[:])
            wot_view = wot.rearrange("(o p) f -> p o f", p=P)
            for nf in range(E // 512):
                wo_sb = wo_pool.tile([P, EC, 512], bf16, tag="wosb")
                nc.sync.dma_start(
                    wo_sb[:], wot_view[:, :, nf * 512:(nf + 1) * 512])
                wo_rs.append(wo_sb)

        def outproj_ops(qh, pool):
            """Out-projection for query-half qh: reads onT[qh] tiles."""
            ops = []
            st = {}
            for qc in range(4 * qh, 4 * qh + 4):
                for nf in range(E // 512):
                    def palloc(qc=qc, nf=nf):
                        st["py"] = pool.tile([P, 512], f32, tag="PROJ",
                                             name=f"py{qc}_{nf}")
                    ops.append(palloc)
                    for hp in range(HP):
                        def pmm(hp=hp, qc=qc, nf=nf):
                            nc.tensor.matmul(
                                st["py"][:],
                                onT[qh][hp][:, (qc % 4) * P:(qc % 4 + 1) * P],
                                wo_rs[nf][:, hp, :],
                                start=(hp == 0), stop=(hp == HP - 1))
                        ops.append(pmm)

                    def pout(qc=qc, nf=nf):
                        y_sb = yp.tile([P, 512], f32, tag="ysb")
                        nc.vector.tensor_add(
                            y_sb[:], st["py"][:],
                            bo_rep[0][:, nf * 512:(nf + 1) * 512])
                        # alternate hwdge queues so the final stores drain
                        # in parallel
                        eng = nc.sync if (qc + nf) % 2 == 0 else nc.scalar
                        eng.dma_start(
                            y[qc * P:(qc + 1) * P, nf * 512:(nf + 1) * 512],
                            y_sb[:])
                    ops.append(pout)
            return ops

        with ExitStack() as actx:
            ps = actx.enter_context(
                tc.tile_pool(name="ps", bufs=2, space="PSUM"))
            wu_p = ps.tile([P, 512], f32, tag="S", name="wu_p")
            for _ in range(16):
                nc.tensor.matmul(wu_p[:], wu[:, 0:P], wu[:],
                                 start=True, stop=True)
            ps_o = actx.enter_context(
                tc.tile_pool(name="ps_o", bufs=3, space="PSUM"))
            xt_pool = actx.enter_context(tc.tile_pool(name="xt", bufs=1))
            w1 = actx.enter_context(tc.tile_pool(name="w1", bufs=1))
            w2 = actx.enter_context(tc.tile_pool(name="w2", bufs=2))
            vp_pool = actx.enter_context(tc.tile_pool(name="vp", bufs=2))
            ut_pool = actx.enter_context(tc.tile_pool(name="ut", bufs=6))

            # xT in SBUF (bf16), 4 token-chunk tiles; queries are chunks 0:2
            xTs = [xt_pool.tile([P, EC, 512], bf16, tag=f"xt{i}",
                            name=f"xT{i}")
                   for i in range(XCH)]
            xt_view = xt_d.rearrange("(o p) t -> p o t", p=P)

            def xtc(ec, t0, t1):
                """View of xT columns t0:t1 (within one 512 chunk) at ec."""
                c = t0 // 512
                assert t1 <= (c + 1) * 512
                return xTs[c][:, ec, t0 - c * 512:t1 - c * 512]

            qt_t, kt_t, vp_t = {}, {}, {}

            def proj_ops(hp, by_chunk=False):
                pre = []
                qb, kb, vb = [], [], []
                st = {}
                # prologue weights ride the Activation hwdge queue so they
                # don't delay the x chunks on the SP queue
                dma_eng = nc.scalar if hp == 0 else nc.sync

                def wload():
                    st["w"] = w2.tile([P, EC, 2, P], bf16, tag="wdma",
                                      name=f"w{hp}")
                    for wi, w_dram in enumerate((wqt, wkt)):
                        dma_eng.dma_start(
                            st["w"][:, :, wi, :],
                            w_dram.rearrange("(o p) f -> p o f", p=P)[
                                :, :, hp * P:(hp + 1) * P])
                pre.append(wload)

                if hp % 2 == 0:
                    def vload():
                        st["wv"] = w2.tile([P, EC, 2 * P], bf16, tag="wdma_v",
                                           name=f"wv{hp}")
                        dma_eng.dma_start(
                            st["wv"][:],
                            wvt.rearrange("(o p) f -> p o f", p=P)[
                                :, :, hp * P:(hp + 2) * P])
                        vp_t[hp // 2] = vp_pool.tile(
                            [P, TC, 4, 65], bf16, tag="vp",
                            name=f"vp{hp // 2}")
                        nc.vector.tensor_copy(
                            vp_t[hp // 2][:, :, :, 64:65],
                            ones_col[:, None, None, :].to_broadcast(
                                [P, TC, 4, 1]))
                    pre.append(vload)

                # QT: two q-halves, each accumulated over ec in own psum
                for nq in range(SQ // 512):
                    blk = []

                    def qalloc(nq=nq):
                        if nq == 0:
                            qt_t[hp] = w2.tile([P, SQ], bf16, tag="qt",
                                               name=f"qt{hp}")
                        st["pq"] = ps_p.tile([P, 512], f32, tag="PROJ",
                                             name=f"pq{hp}_{nq}")
                    blk.append(qalloc)
                    for ec in range(EC):
                        def qmm(ec=ec, nq=nq):
                            nc.tensor.matmul(
                                st["pq"][:], st["w"][:, ec, 0],
                                xtc(ec, nq * 512, (nq + 1) * 512),
                                start=(ec == 0), stop=(ec == EC - 1))
                        blk.append(qmm)

                    def qcopy(nq=nq):
                        nc.vector.tensor_copy(
                            qt_t[hp][:, nq * 512:(nq + 1) * 512], st["pq"][:])
                    blk.append(qcopy)
                    qb.append(blk)

                # KT: four 512-chunks
                for nk in range(S // 512):
                    blk = []

                    def kalloc(nk=nk):
                        if nk == 0:
                            kt_t[hp] = w2.tile([P, S], bf16, tag="kt",
                                               name=f"kt{hp}")
                        st["pk"] = ps_p.tile([P, 512], f32, tag="PROJ",
                                             name=f"pk{hp}_{nk}")
                    blk.append(kalloc)
                    for ec in range(EC):
                        def kmm(ec=ec, nk=nk):
                            nc.tensor.matmul(
                                st["pk"][:], st["w"][:, ec, 1],
                                xtc(ec, nk * 512, (nk + 1) * 512),
                                start=(ec == 0), stop=(ec == EC - 1))
                        blk.append(kmm)

                    def kcopy(nk=nk):
                        nc.vector.tensor_copy(
                            kt_t[hp][:, nk * 512:(nk + 1) * 512], st["pk"][:])
                    blk.append(kcopy)
                    kb.append(blk)

                # V for the pair (hp, hp+1) on even hp: out free dim 256
                if hp % 2 == 0:
                    for tc_i in range(TC):
                        blk = []

                        def valloc(tc_i=tc_i):
                            st["pv"] = ps_p.tile([P, 512], f32, tag="PROJ",
                                                 name=f"pv{hp}_{tc_i}")
                        blk.append(valloc)
                        for ec in range(EC):
                            def vmm(ec=ec, tc_i=tc_i):
                                nc.tensor.matmul(
                                    st["pv"][:, :256],
                                    xtc(ec, tc_i * P, (tc_i + 1) * P),
                                    st["wv"][:, ec, :],
                                    start=(ec == 0), stop=(ec == EC - 1))
                            blk.append(vmm)

                        def vcopy(tc_i=tc_i):
                            nc.vector.tensor_copy(
                                vp_t[hp // 2][:, tc_i, :, 0:64],
                                st["pv"][:, :256].rearrange(
                                    "p (h d) -> p h d", h=4))
                        blk.append(vcopy)
                        vb.append(blk)

                ops = []
                if by_chunk:
                    # hp0 runs against the in-flight x DMAs: emit Q/K for
                    # each chunk as it lands, and use V blocks (which only
                    # need already-landed chunks) as filler so the PE never
                    # outpaces the DMA queue
                    order = [qb[0], kb[0], qb[1], kb[1],
                             vb[0], vb[1], vb[2], vb[3],
                             kb[2], vb[4], vb[5], vb[6], vb[7],
                             kb[3]] + vb[8:]
                    for blk in order:
                        ops += blk
                else:
                    # V blocks before the last two K chunks: the next
                    # block's first probs@V needs the final V copy, while
                    # K chunks 2-3 aren't read until its kc=8
                    for blk in qb + kb[:2] + vb + kb[2:]:
                        ops += blk
                return pre, ops

            # prologue: head-pair 0's weight DMAs (Activation queue), then
            # x DMAs, then head-pair 1's weight DMAs, then hp0's projections
            pre0, rest0 = proj_ops(0, by_chunk=True)
            pre_d, rest_d = {}, {}
            pre_d[1], rest_d[1] = proj_ops(1)
            for op in pre0:
                op()
            for i in range(XCH):
                nc.sync.dma_start(xTs[i][:], xt_view[:, :, i * 512:(i + 1) * 512])
            for op in pre_d[1]:
                op()
            for op in rest0:
                op()

            for hp in range(HP):
                qt, kt = qt_t[hp], kt_t[hp]
                vp = vp_t[hp // 2]
                ha, hb = 2 * (hp % 2), 2 * (hp % 2) + 1
                if hp + 1 < HP:
                    # weight DMAs for head-pair hp+2 go out one block early
                    # (double-buffered w tiles), so hp+1's projections never
                    # wait on their weights mid-block
                    allops = []
                    if hp + 2 < HP:
                        pre_d[hp + 2], rest_d[hp + 2] = proj_ops(hp + 2)
                        allops += pre_d[hp + 2]
                    allops += rest_d[hp + 1]
                    nxt = {0: allops[:len(allops) // 2],
                           1: allops[len(allops) // 2:]}
                else:
                    # last head-pair: interleave first-half out-projection
                    # into the second query-half's attention stream
                    nxt = {0: [], 1: outproj_ops(0, ps_p)}

                for qh in range(2):
                    ops_q = nxt[qh]
                    n_emit = 0
                    po_a = ps_o.tile([65, 512], f32, tag="po")
                    po_b = ps_o.tile([65, 512], f32, tag="po")
                    qs = slice(qh * 512, (qh + 1) * 512)

                    def pv(kc, ut):
                        nc.tensor.matmul(
                            po_a[:], vp[:, kc, ha], ut[:, 0:512],
                            start=(kc == 0), stop=(kc == TC - 1))
                        nc.tensor.matmul(
                            po_b[:], vp[:, kc, hb], ut[:, 512:1024],
                            start=(kc == 0), stop=(kc == TC - 1))

                    prev = []
                    for kc in range(TC):
                        sc = ps.tile([P, 1024], f32, tag="S")
                        nc.tensor.matmul(
                            sc[:, 0:512], kt[0:64, kc * P:(kc + 1) * P],
                            qt[0:64, qs], start=True, stop=True)
                        nc.tensor.matmul(
                            sc[:, 512:1024], kt[64:128, kc * P:(kc + 1) * P],
                            qt[64:128, qs], start=True, stop=True)
                        ut = ut_pool.tile([P, 1024], bf16, tag="ut")
                        nc.scalar.activation(
                            ut[:], sc[:], AF.Exp, scale=0.125)
                        # emit probs@V two kc behind the scores: the PE
                        # always has exp-independent work queued while ACT
                        # computes
                        if len(prev) == 3:
                            pv(*prev.pop(0))
                        prev.append((kc, ut))
                        # interleave pipelined work
                        target = len(ops_q) * (kc + 1) // TC
                        while n_emit < target:
                            ops_q[n_emit]()
                            n_emit += 1
                    for pr in prev:
                        pv(*pr)
                    assert n_emit == len(ops_q)

                    # evacuate po immediately (frees the PSUM banks; a DVE
                    # op may read PSUM across partitions, so head b lands at
                    # partitions 64:128), then normalize: row 64 of each po
                    # is the softmax denominator. partition_broadcast only
                    # writes correctly from base 0: broadcast into a full
                    # tile, slice at read time.
                    po_s = w1.tile([P, 512], f32, tag="po_s")
                    nc.vector.tensor_copy(po_s[0:64, :], po_a[0:64, :])
                    rcp_a = w1.tile([1, 512], f32, tag="rcp_a")
                    nc.vector.reciprocal(rcp_a[:], po_a[64:65, :])
                    nc.vector.tensor_copy(po_s[64:128, :], po_b[0:64, :])
                    rcp_b = w1.tile([1, 512], f32, tag="rcp_b")
                    nc.vector.reciprocal(rcp_b[:], po_b[64:65, :])
                    brec_a = w1.tile([P, 512], f32, tag="brec_a")
                    nc.gpsimd.partition_broadcast(brec_a[:], rcp_a[:])
                    brec_b = w1.tile([P, 512], f32, tag="brec_b")
                    nc.gpsimd.partition_broadcast(brec_b[:], rcp_b[:])
                    nc.vector.tensor_mul(
                        onT[qh][hp][0:64, :], po_s[0:64, :], brec_a[0:64, :])
                    nc.vector.tensor_mul(
                        onT[qh][hp][64:128, :], po_s[64:128, :],
                        brec_b[64:128, :])

                # prefetch out-projection weights mid-attention
                if hp == 2:
                    prefetch_wo()

        # ---- output projection tail: second query-half ----
        with ExitStack() as dctx:
            ps_t = dctx.enter_context(
                tc.tile_pool(name="ps_t", bufs=3, space="PSUM"))
            for op in outproj_ops(1, ps_t):
                op()


def _build_kernel(reps=1):
    nc = bacc.Bacc("TRN2", target_bir_lowering=False, debug=False,
                   num_devices=N_CORES)
    xt_d = nc.dram_tensor("xt", [E, S], dt.bfloat16,
                          kind="ExternalInput").ap()
    wqt = nc.dram_tensor("wqt", [E, E], dt.bfloat16,
                         kind="ExternalInput").ap()
    wkt = nc.dram_tensor("wkt", [E, E], dt.bfloat16,
                         kind="ExternalInput").ap()
    wvt = nc.dram_tensor("wvt", [E, E], dt.bfloat16,
                         kind="ExternalInput").ap()
    wot = nc.dram_tensor("wot", [E, E], dt.bfloat16,
                         kind="ExternalInput").ap()
    bo = nc.dram_tensor("bo", [1, E], dt.float32, kind="ExternalInput").ap()
    y = nc.dram_tensor("y", [SQ, E], dt.float32, kind="ExternalOutput").ap()

    with tile.TileContext(nc) as tc:
        for _ in range(reps):
            _emit(nc, tc, xt_d, wqt, wkt, wvt, wot, bo, y)
    nc.compile()
    return nc


_NC_CACHE = None


def _bf16(a):
    import ml_dtypes
    return np.ascontiguousarray(np.asarray(a, np.float32).astype(
        ml_dtypes.bfloat16))


def make_in_maps(x, Wq, Wk, Wv, Wo, bo):
    x = np.asarray(x, np.float32)
    wqt = _bf16(np.asarray(Wq, np.float32).T)
    wkt = _bf16(np.asarray(Wk, np.float32).T)
    wvt = _bf16(np.asarray(Wv, np.float32).T)
    wot = _bf16(np.asarray(Wo, np.float32).T)
    bo_ = np.ascontiguousarray(np.asarray(bo, np.float32).reshape(1, E))

    in_maps = []
    for c in range(N_CORES):
        b, half = c // 2, c % 2
        # rotate so this core's query block is rows 0:SQ (keys are a
        # permutation of the sequence -- attention is invariant to key order)
        xt_rot = _bf16(np.roll(x[b], -half * SQ, axis=0).T)
        in_maps.append({"xt": xt_rot, "wqt": wqt, "wkt": wkt, "wvt": wvt,
                        "wot": wot, "bo": bo_})
    return in_maps


def get_nc(reps=1):
    global _NC_CACHE
    if _NC_CACHE is None:
        _NC_CACHE = {}
    if reps not in _NC_CACHE:
        _NC_CACHE[reps] = _build_kernel(reps)
    return _NC_CACHE[reps]


def kernel(x, Wq, Wk, Wv, Wo, bo):
    nc = get_nc()
    in_maps = make_in_maps(x, Wq, Wk, Wv, Wo, bo)
    res = run_bass_kernel_spmd(nc, in_maps, core_ids=list(range(N_CORES)))
    out = np.empty((B, S, E), np.float32)
    for c in range(N_CORES):
        b, half = c // 2, c % 2
        out[b, half * SQ:(half + 1) * SQ, :] = res.results[c]["y"]
    return out


# revision 28
# speedup vs baseline: 1.0705x; 1.0162x over previous
"""Multi-head attention (B=4, S=2048, E=1024, H=16, D=64) on 8 TRN2 cores.

Sharding: core c handles batch b = c//2, query half = c%2 (1024 queries).
Each core computes K/V over its batch's full sequence (duplicated between the
two half-cores of a batch -- cheaper at these sizes than any collective),
attention for all 16 heads over its 1024 queries, and the output projection
for its output chunk. Outputs are disjoint -> host gather is concatenation.

The host rotates each core's sequence so its query block is always rows
0:1024 (attention is permutation-invariant over keys), pre-transposes the
weights and activations (pure layout prep) so the e-contraction projections
have e on partitions, and converts everything to bf16 (rel-err budget 2e-2;
bf16 lands ~3e-3).

bf16 operands keep the PE at full rate, enable Fast Weight Load, halve DMA
bytes, and remove all dtype-staging copies: DMAs land directly in the SBUF
tiles the matmuls read. PSUM accumulation stays fp32.

Scores matmuls use contraction 64 (head dim) at base partitions 0/64, which
bass auto-lowers to 64x128 row tiles (0,0)/(64,0) -- on hardware the two
head matmuls run concurrently in the two half-arrays.

Schedule: the first head-pair's weight DMAs go out before the x DMAs (so the
PE starts ~5us in, not ~16us); head-pair hp+1's projection instructions are
interleaved into hp's attention stream; the out-projection weights prefetch
mid-attention; the first query-half's out-projection interleaves into the
last head-pair's attention; softmax state is copied out of PSUM immediately
so the po banks recycle without waiting on the normalize chain.
"""

from contextlib import ExitStack

import numpy as np

import concourse.tile as tile
from concourse import bacc, mybir
from concourse.bass_utils import run_bass_kernel_spmd

dt = mybir.dt
AF = mybir.ActivationFunctionType

B, S, E, H, D = 4, 2048, 1024, 16, 64
N_CORES = 8
SQ = 1024          # queries per core
P = 128
EC = E // P        # 8 e-chunks
TC = S // P        # 16 t-chunks (keys)
QC = SQ // P       # 8 query chunks
HP = H // 2        # 8 head-pairs
XCH = 4            # xT token chunks (512 tokens each)


def _emit(nc, tc, xt_d, wqt, wkt, wvt, wot, bo, y):
    f32, bf16 = dt.float32, dt.bfloat16

    with ExitStack() as ctx:
        const = ctx.enter_context(tc.tile_pool(name="const", bufs=1))
        ps_p = ctx.enter_context(
            tc.tile_pool(name="ps_p", bufs=2, space="PSUM"))
        on_pool = ctx.enter_context(tc.tile_pool(name="on", bufs=1))
        wo_pool = ctx.enter_context(tc.tile_pool(name="wo", bufs=2))
        yp = ctx.enter_context(tc.tile_pool(name="yp", bufs=4))

        ones_col = const.tile([P, 1], bf16)
        nc.vector.memset(ones_col[:], 1.0)

        # warm the PE (p-state / HAM) with dependency-free matmuls while the
        # first DMAs are in flight
        wu = const.tile([P, 512], bf16)
        nc.vector.memset(wu[:], 0.0)

        # attention output, [e, q] layout: partition j of tile (qh, hp) is
        # e-row hp*128+j (head 2hp on partitions 0:64, 2hp+1 on 64:128).
        # One tile per (query-half, head-pair) so out-projection reads only
        # depend on the normalizes that actually produced them.
        onT = [[on_pool.tile([P, 512], bf16, tag=f"on{qh}_{hp}",
                             name=f"onT{qh}_{hp}")
                for hp in range(HP)]
               for qh in range(2)]
        wo_rs = []
        bo_rep = [None]

        def prefetch_wo():
            bo_one = wo_pool.tile([1, E], f32, tag="bo1")
            nc.sync.dma_start(bo_one[:], bo[:])
            bo_rep[0] = wo_pool.tile([P, E], f32, tag="bor", name="bo_rep")
            nc.gpsimd.partition_broadcast(bo_rep[0][:], bo_one[:])
            wot_view = wot.rearrange("(o p) f -> p o f", p=P)
            for nf in range(E // 512):
                wo_sb = wo_pool.tile([P, EC, 512], bf16, tag="wosb")
                nc.sync.dma_start(
                    wo_sb[:], wot_view[:, :, nf * 512:(nf + 1) * 512])
                wo_rs.append(wo_sb)

        def outproj_ops(qh, pool):
            """Out-projection for query-half qh: reads onT[qh] tiles."""
            ops = []
            st = {}
            for qc in range(4 * qh, 4 * qh + 4):
                for nf in range(E // 512):
                    def palloc(qc=qc, nf=nf):
                        st["py"] = pool.tile([P, 512], f32, tag="PROJ",
                                             name=f"py{qc}_{nf}")
                    ops.append(palloc)
                    for hp in range(HP):
                        def pmm(hp=hp, qc=qc, nf=nf):
                            nc.tensor.matmul(
                                st["py"][:],
                                onT[qh][hp][:, (qc % 4) * P:(qc % 4 + 1) * P],
                                wo_rs[nf][:, hp, :],
                                start=(hp == 0), stop=(hp == HP - 1))
                        ops.append(pmm)

                    def pout(qc=qc, nf=nf):
                        y_sb = yp.tile([P, 512], f32, tag="ysb")
                        nc.vector.tensor_add(
                            y_sb[:], st["py"][:],
                            bo_rep[0][:, nf * 512:(nf + 1) * 512])
                        # alternate hwdge queues so the final stores drain
                        # in parallel
                        eng = nc.sync if (qc + nf) % 2 == 0 else nc.scalar
                        eng.dma_start(
                            y[qc * P:(qc + 1) * P, nf * 512:(nf + 1) * 512],
                            y_sb[:])
                    ops.append(pout)
            return ops

        with ExitStack() as actx:
            ps = actx.enter_context(
                tc.tile_pool(name="ps", bufs=2, space="PSUM"))
            wu_p = ps.tile([P, 512], f32, tag="S", name="wu_p")
            for _ in range(16):
                nc.tensor.matmul(wu_p[:], wu[:, 0:P], wu[:],
                                 start=True, stop=True)
            ps_o = actx.enter_context(
                tc.tile_pool(name="ps_o", bufs=2, space="PSUM"))
            xt_pool = actx.enter_context(tc.tile_pool(name="xt", bufs=1))
            w1 = actx.enter_context(tc.tile_pool(name="w1", bufs=1))
            w2 = actx.enter_context(tc.tile_pool(name="w2", bufs=2))
            vp_pool = actx.enter_context(tc.tile_pool(name="vp", bufs=2))
            ut_pool = actx.enter_context(tc.tile_pool(name="ut", bufs=6))

            # xT in SBUF (bf16), 4 token-chunk tiles; queries are chunks 0:2
            xTs = [xt_pool.tile([P, EC, 512], bf16, tag=f"xt{i}",
                            name=f"xT{i}")
                   for i in range(XCH)]
            xt_view = xt_d.rearrange("(o p) t -> p o t", p=P)

            def xtc(ec, t0, t1):
                """View of xT columns t0:t1 (within one 512 chunk) at ec."""
                c = t0 // 512
                assert t1 <= (c + 1) * 512
                return xTs[c][:, ec, t0 - c * 512:t1 - c * 512]

            qt_t, kt_t, vp_t = {}, {}, {}

            def proj_ops(hp, by_chunk=False):
                pre = []
                qb, kb, vb = [], [], []
                st = {}
                # prologue weights ride the Activation hwdge queue so they
                # don't delay the x chunks on the SP queue
                dma_eng = nc.scalar if hp == 0 else nc.sync

                def wload():
                    st["w"] = w2.tile([P, EC, 2, P], bf16, tag="wdma",
                                      name=f"w{hp}")
                    for wi, w_dram in enumerate((wqt, wkt)):
                        dma_eng.dma_start(
                            st["w"][:, :, wi, :],
                            w_dram.rearrange("(o p) f -> p o f", p=P)[
                                :, :, hp * P:(hp + 1) * P])
                pre.append(wload)

                if hp % 2 == 0:
                    def vload():
                        st["wv"] = w2.tile([P, EC, 2 * P], bf16, tag="wdma_v",
                                           name=f"wv{hp}")
                        dma_eng.dma_start(
                            st["wv"][:],
                            wvt.rearrange("(o p) f -> p o f", p=P)[
                                :, :, hp * P:(hp + 2) * P])
                        vp_t[hp // 2] = vp_pool.tile(
                            [P, TC, 4, 65], bf16, tag="vp",
                            name=f"vp{hp // 2}")
                        nc.vector.tensor_copy(
                            vp_t[hp // 2][:, :, :, 64:65],
                            ones_col[:, None, None, :].to_broadcast(
                                [P, TC, 4, 1]))
                    pre.append(vload)

                # QT: two q-halves, each accumulated over ec in own psum
                for nq in range(SQ // 512):
                    blk = []

                    def qalloc(nq=nq):
                        if nq == 0:
                            qt_t[hp] = w2.tile([P, SQ], bf16, tag="qt",
                                               name=f"qt{hp}")
                        st["pq"] = ps_p.tile([P, 512], f32, tag="PROJ",
                                             name=f"pq{hp}_{nq}")
                    blk.append(qalloc)
                    for ec in range(EC):
                        def qmm(ec=ec, nq=nq):
                            nc.tensor.matmul(
                                st["pq"][:], st["w"][:, ec, 0],
                                xtc(ec, nq * 512, (nq + 1) * 512),
                                start=(ec == 0), stop=(ec == EC - 1))
                        blk.append(qmm)

                    def qcopy(nq=nq):
                        nc.vector.tensor_copy(
                            qt_t[hp][:, nq * 512:(nq + 1) * 512], st["pq"][:])
                    blk.append(qcopy)
                    qb.append(blk)

                # KT: four 512-chunks
                for nk in range(S // 512):
                    blk = []

                    def kalloc(nk=nk):
                        if nk == 0:
                            kt_t[hp] = w2.tile([P, S], bf16, tag="kt",
                                               name=f"kt{hp}")
                        st["pk"] = ps_p.tile([P, 512], f32, tag="PROJ",
                                             name=f"pk{hp}_{nk}")
                    blk.append(kalloc)
                    for ec in range(EC):
                        def kmm(ec=ec, nk=nk):
                            nc.tensor.matmul(
                                st["pk"][:], st["w"][:, ec, 1],
                                xtc(ec, nk * 512, (nk + 1) * 512),
                                start=(ec == 0), stop=(ec == EC - 1))
                        blk.append(kmm)

                    def kcopy(nk=nk):
                        nc.vector.tensor_copy(
                            kt_t[hp][:, nk * 512:(nk + 1) * 512], st["pk"][:])
                    blk.append(kcopy)
                    kb.append(blk)

                # V for the pair (hp, hp+1) on even hp: out free dim 256
                if hp % 2 == 0:
                    for tc_i in range(TC):
                        blk = []

                        def valloc(tc_i=tc_i):
                            st["pv"] = ps_p.tile([P, 512], f32, tag="PROJ",
                                                 name=f"pv{hp}_{tc_i}")
                        blk.append(valloc)
                        for ec in range(EC):
                            def vmm(ec=ec, tc_i=tc_i):
                                nc.tensor.matmul(
                                    st["pv"][:, :256],
                                    xtc(ec, tc_i * P, (tc_i + 1) * P),
                                    st["wv"][:, ec, :],
                                    start=(ec == 0), stop=(ec == EC - 1))
                            blk.append(vmm)

                        def vcopy(tc_i=tc_i):
                            nc.vector.tensor_copy(
                                vp_t[hp // 2][:, tc_i, :, 0:64],
                                st["pv"][:, :256].rearrange(
                                    "p (h d) -> p h d", h=4))
                        blk.append(vcopy)
                        vb.append(blk)

                ops = []
                if by_chunk:
                    # hp0 runs against the in-flight x DMAs: emit Q/K for
                    # each chunk as it lands, and use V blocks (which only
                    # need already-landed chunks) as filler so the PE never
                    # outpaces the DMA queue
                    order = [qb[0], kb[0], qb[1], kb[1],
                             vb[0], vb[1], vb[2], vb[3],
                             kb[2], vb[4], vb[5], vb[6], vb[7],
                             kb[3]] + vb[8:]
                    for blk in order:
                        ops += blk
                else:
                    # V blocks before the last two K chunks: the next
                    # block's first probs@V needs the final V copy, while
                    # K chunks 2-3 aren't read until its kc=8
                    for blk in qb + kb[:2] + vb + kb[2:]:
                        ops += blk
                return pre, ops

            # prologue: head-pair 0's weight DMAs (Activation queue), then
            # x DMAs, then head-pair 1's weight DMAs, then hp0's projections
            pre0, rest0 = proj_ops(0, by_chunk=True)
            pre_d, rest_d = {}, {}
            pre_d[1], rest_d[1] = proj_ops(1)
            for op in pre0:
                op()
            for i in range(XCH):
                nc.sync.dma_start(xTs[i][:], xt_view[:, :, i * 512:(i + 1) * 512])
            for op in pre_d[1]:
                op()
            for op in rest0:
                op()

            for hp in range(HP):
                qt, kt = qt_t[hp], kt_t[hp]
                vp = vp_t[hp // 2]
                ha, hb = 2 * (hp % 2), 2 * (hp % 2) + 1
                if hp + 1 < HP:
                    # weight DMAs for head-pair hp+2 go out one block early
                    # (double-buffered w tiles), so hp+1's projections never
                    # wait on their weights mid-block
                    allops = []
                    if hp + 2 < HP:
                        pre_d[hp + 2], rest_d[hp + 2] = proj_ops(hp + 2)
                        allops += pre_d[hp + 2]
                    allops += rest_d[hp + 1]
                    nxt = {0: allops[:len(allops) // 2],
                           1: allops[len(allops) // 2:]}
                else:
                    # last head-pair: interleave first-half out-projection
                    # into the second query-half's attention stream
                    nxt = {0: [], 1: outproj_ops(0, ps_p)}

                for qh in range(2):
                    ops_q = nxt[qh]
                    n_emit = 0
                    po_a = ps_o.tile([65, 512], f32, tag="po")
                    po_b = ps_o.tile([65, 512], f32, tag="po")
                    qs = slice(qh * 512, (qh + 1) * 512)

                    def pv(kc, ut):
                        nc.tensor.matmul(
                            po_a[:], vp[:, kc, ha], ut[:, 0:512],
                            start=(kc == 0), stop=(kc == TC - 1))
                        nc.tensor.matmul(
                            po_b[:], vp[:, kc, hb], ut[:, 512:1024],
                            start=(kc == 0), stop=(kc == TC - 1))

                    prev = []
                    for kc in range(TC):
                        sc = ps.tile([P, 1024], f32, tag="S")
                        nc.tensor.matmul(
                            sc[:, 0:512], kt[0:64, kc * P:(kc + 1) * P],
                            qt[0:64, qs], start=True, stop=True)
                        nc.tensor.matmul(
                            sc[:, 512:1024], kt[64:128, kc * P:(kc + 1) * P],
                            qt[64:128, qs], start=True, stop=True)
                        ut = ut_pool.tile([P, 1024], bf16, tag="ut")
                        nc.scalar.activation(
                            ut[:], sc[:], AF.Exp, scale=0.125)
                        # emit probs@V two kc behind the scores: the PE
                        # always has exp-independent work queued while ACT
                        # computes
                        if len(prev) == 3:
                            pv(*prev.pop(0))
                        prev.append((kc, ut))
                        # interleave pipelined work
                        target = len(ops_q) * (kc + 1) // TC
                        while n_emit < target:
                            ops_q[n_emit]()
                            n_emit += 1
                    for pr in prev:
                        pv(*pr)
                    assert n_emit == len(ops_q)

                    # evacuate po immediately (frees the PSUM banks; a DVE
                    # op may read PSUM across partitions, so head b lands at
                    # partitions 64:128), then normalize: row 64 of each po
                    # is the softmax denominator. partition_broadcast only
                    # writes correctly from base 0: broadcast into a full
                    # tile, slice at read time.
                    po_s = w1.tile([P, 512], f32, tag="po_s")
                    nc.vector.tensor_copy(po_s[0:64, :], po_a[0:64, :])
                    rcp_a = w1.tile([1, 512], f32, tag="rcp_a")
                    nc.vector.reciprocal(rcp_a[:], po_a[64:65, :])
                    nc.vector.tensor_copy(po_s[64:128, :], po_b[0:64, :])
                    rcp_b = w1.tile([1, 512], f32, tag="rcp_b")
                    nc.vector.reciprocal(rcp_b[:], po_b[64:65, :])
                    brec_a = w1.tile([P, 512], f32, tag="brec_a")
                    nc.gpsimd.partition_broadcast(brec_a[:], rcp_a[:])
                    brec_b = w1.tile([P, 512], f32, tag="brec_b")
                    nc.gpsimd.partition_broadcast(brec_b[:], rcp_b[:])
                    nc.vector.tensor_mul(
                        onT[qh][hp][0:64, :], po_s[0:64, :], brec_a[0:64, :])
                    nc.vector.tensor_mul(
                        onT[qh][hp][64:128, :], po_s[64:128, :],
                        brec_b[64:128, :])

                # prefetch out-projection weights mid-attention
                if hp == 2:
                    prefetch_wo()

        # ---- output projection tail: second query-half ----
        with ExitStack() as dctx:
            ps_t = dctx.enter_context(
                tc.tile_pool(name="ps_t", bufs=3, space="PSUM"))
            for op in outproj_ops(1, ps_t):
                op()


def _build_kernel(reps=1):
    nc = bacc.Bacc("TRN2", target_bir_lowering=False, debug=False,
                   num_devices=N_CORES)
    xt_d = nc.dram_tensor("xt", [E, S], dt.bfloat16,
                          kind="ExternalInput").ap()
    wqt = nc.dram_tensor("wqt", [E, E], dt.bfloat16,
                         kind="ExternalInput").ap()
    wkt = nc.dram_tensor("wkt", [E, E], dt.bfloat16,
                         kind="ExternalInput").ap()
    wvt = nc.dram_tensor("wvt", [E, E], dt.bfloat16,
                         kind="ExternalInput").ap()
    wot = nc.dram_tensor("wot", [E, E], dt.bfloat16,
                         kind="ExternalInput").ap()
    bo = nc.dram_tensor("bo", [1, E], dt.float32, kind="ExternalInput").ap()
    y = nc.dram_tensor("y", [SQ, E], dt.float32, kind="ExternalOutput").ap()

    with tile.TileContext(nc) as tc:
        for _ in range(reps):
            _emit(nc, tc, xt_d, wqt, wkt, wvt, wot, bo, y)
    nc.compile()
    return nc


_NC_CACHE = None


def _bf16(a):
    import ml_dtypes
    return np.ascontiguousarray(np.asarray(a, np.float32).astype(
        ml_dtypes.bfloat16))


def make_in_maps(x, Wq, Wk, Wv, Wo, bo):
    x = np.asarray(x, np.float32)
    wqt = _bf16(np.asarray(Wq, np.float32).T)
    wkt = _bf16(np.asarray(Wk, np.float32).T)
    wvt = _bf16(np.asarray(Wv, np.float32).T)
    wot = _bf16(np.asarray(Wo, np.float32).T)
    bo_ = np.ascontiguousarray(np.asarray(bo, np.float32).reshape(1, E))

    in_maps = []
    for c in range(N_CORES):
        b, half = c // 2, c % 2
        # rotate so this core's query block is rows 0:SQ (keys are a
        # permutation of the sequence -- attention is invariant to key order)
        xt_rot = _bf16(np.roll(x[b], -half * SQ, axis=0).T)
        in_maps.append({"xt": xt_rot, "wqt": wqt, "wkt": wkt, "wvt": wvt,
                        "wot": wot, "bo": bo_})
    return in_maps


def get_nc(reps=1):
    global _NC_CACHE
    if _NC_CACHE is None:
        _NC_CACHE = {}
    if reps not in _NC_CACHE:
        _NC_CACHE[reps] = _build_kernel(reps)
    return _NC_CACHE[reps]


def kernel(x, Wq, Wk, Wv, Wo, bo):
    nc = get_nc()
    in_maps = make_in_maps(x, Wq, Wk, Wv, Wo, bo)
    res = run_bass_kernel_spmd(nc, in_maps, core_ids=list(range(N_CORES)))
    out = np.empty((B, S, E), np.float32)
    for c in range(N_CORES):
        b, half = c // 2, c % 2
        out[b, half * SQ:(half + 1) * SQ, :] = res.results[c]["y"]
    return out


# revision 33
# speedup vs baseline: 1.0750x; 1.0041x over previous
"""Multi-head attention (B=4, S=2048, E=1024, H=16, D=64) on 8 TRN2 cores.

Sharding: core c handles batch b = c//2, query half = c%2 (1024 queries).
Each core computes K/V over its batch's full sequence (duplicated between the
two half-cores of a batch -- cheaper at these sizes than any collective),
attention for all 16 heads over its 1024 queries, and the output projection
for its output chunk. Outputs are disjoint -> host gather is concatenation.

The host rotates each core's sequence so its query block is always rows
0:1024 (attention is permutation-invariant over keys), pre-transposes the
weights and activations (pure layout prep) so the e-contraction projections
have e on partitions, and converts everything to bf16 (rel-err budget 2e-2;
bf16 lands ~3e-3).

bf16 operands keep the PE at full rate, enable Fast Weight Load, halve DMA
bytes, and remove all dtype-staging copies: DMAs land directly in the SBUF
tiles the matmuls read. PSUM accumulation stays fp32.

Scores matmuls use contraction 64 (head dim) at base partitions 0/64, which
bass auto-lowers to 64x128 row tiles (0,0)/(64,0) -- on hardware the two
head matmuls run concurrently in the two half-arrays.

Schedule: the first head-pair's weight DMAs go out before the x DMAs (so the
PE starts ~5us in, not ~16us); head-pair hp+1's projection instructions are
interleaved into hp's attention stream; the out-projection weights prefetch
mid-attention; the first query-half's out-projection interleaves into the
last head-pair's attention; softmax state is copied out of PSUM immediately
so the po banks recycle without waiting on the normalize chain.
"""

from contextlib import ExitStack

import numpy as np

import concourse.tile as tile
from concourse import bacc, mybir
from concourse.bass_utils import run_bass_kernel_spmd

dt = mybir.dt
AF = mybir.ActivationFunctionType

B, S, E, H, D = 4, 2048, 1024, 16, 64
N_CORES = 8
SQ = 1024          # queries per core
P = 128
EC = E // P        # 8 e-chunks
TC = S // P        # 16 t-chunks (keys)
QC = SQ // P       # 8 query chunks
HP = H // 2        # 8 head-pairs
XCH = 4            # xT token chunks (512 tokens each)


def _emit(nc, tc, xt_d, wqt, wkt, wvt, wot, bo, y):
    f32, bf16 = dt.float32, dt.bfloat16

    with ExitStack() as ctx:
        const = ctx.enter_context(tc.tile_pool(name="const", bufs=1))
        ps_p = ctx.enter_context(
            tc.tile_pool(name="ps_p", bufs=2, space="PSUM"))
        on_pool = ctx.enter_context(tc.tile_pool(name="on", bufs=1))
        wo_pool = ctx.enter_context(tc.tile_pool(name="wo", bufs=2))
        yp = ctx.enter_context(tc.tile_pool(name="yp", bufs=4))

        ones_col = const.tile([P, 1], bf16)
        nc.vector.memset(ones_col[:], 1.0)

        # warm the PE (p-state / HAM) with dependency-free matmuls while the
        # first DMAs are in flight
        wu = const.tile([P, 512], bf16)
        nc.vector.memset(wu[:], 0.0)

        # attention output, [e, q] layout: partition j of tile (qh, hp) is
        # e-row hp*128+j (head 2hp on partitions 0:64, 2hp+1 on 64:128).
        # One tile per (query-half, head-pair) so out-projection reads only
        # depend on the normalizes that actually produced them.
        onT = [[on_pool.tile([P, 512], bf16, tag=f"on{qh}_{hp}",
                             name=f"onT{qh}_{hp}")
                for hp in range(HP)]
               for qh in range(2)]
        wo_rs = []
        bo_rep = [None]

        def prefetch_wo():
            bo_one = wo_pool.tile([1, E], f32, tag="bo1")
            nc.sync.dma_start(bo_one[:], bo[:])
            bo_rep[0] = wo_pool.tile([P, E], f32, tag="bor", name="bo_rep")
            nc.gpsimd.partition_broadcast(bo_rep[0][:], bo_one[:])
            wot_view = wot.rearrange("(o p) f -> p o f", p=P)
            for nf in range(E // 512):
                wo_sb = wo_pool.tile([P, EC, 512], bf16, tag="wosb")
                nc.sync.dma_start(
                    wo_sb[:], wot_view[:, :, nf * 512:(nf + 1) * 512])
                wo_rs.append(wo_sb)

        def outproj_ops(qh, pool):
            """Out-projection for query-half qh: reads onT[qh] tiles."""
            ops = []
            st = {}
            for qc in range(4 * qh, 4 * qh + 4):
                for nf in range(E // 512):
                    def palloc(qc=qc, nf=nf):
                        st["py"] = pool.tile([P, 512], f32, tag="PROJ",
                                             name=f"py{qc}_{nf}")
                    ops.append(palloc)
                    for hp in range(HP):
                        def pmm(hp=hp, qc=qc, nf=nf):
                            nc.tensor.matmul(
                                st["py"][:],
                                onT[qh][hp][:, (qc % 4) * P:(qc % 4 + 1) * P],
                                wo_rs[nf][:, hp, :],
                                start=(hp == 0), stop=(hp == HP - 1))
                        ops.append(pmm)

                    def pout(qc=qc, nf=nf):
                        y_sb = yp.tile([P, 512], f32, tag="ysb")
                        nc.vector.tensor_add(
                            y_sb[:], st["py"][:],
                            bo_rep[0][:, nf * 512:(nf + 1) * 512])
                        # alternate hwdge queues so the final stores drain
                        # in parallel
                        eng = nc.sync if (qc + nf) % 2 == 0 else nc.scalar
                        eng.dma_start(
                            y[qc * P:(qc + 1) * P, nf * 512:(nf + 1) * 512],
                            y_sb[:])
                    ops.append(pout)
            return ops

        with ExitStack() as actx:
            ps = actx.enter_context(
                tc.tile_pool(name="ps", bufs=2, space="PSUM"))
            wu_p = ps.tile([P, 512], f32, tag="S", name="wu_p")
            for _ in range(16):
                nc.tensor.matmul(wu_p[:], wu[:, 0:P], wu[:],
                                 start=True, stop=True)
            ps_o = actx.enter_context(
                tc.tile_pool(name="ps_o", bufs=2, space="PSUM"))
            xt_pool = actx.enter_context(tc.tile_pool(name="xt", bufs=1))
            w1 = actx.enter_context(tc.tile_pool(name="w1", bufs=1))
            w2 = actx.enter_context(tc.tile_pool(name="w2", bufs=2))
            vp_pool = actx.enter_context(tc.tile_pool(name="vp", bufs=2))
            ut_pool = actx.enter_context(tc.tile_pool(name="ut", bufs=6))

            # xT in SBUF (bf16), 4 token-chunk tiles; queries are chunks 0:2
            xTs = [xt_pool.tile([P, EC, 512], bf16, tag=f"xt{i}",
                            name=f"xT{i}")
                   for i in range(XCH)]
            xt_view = xt_d.rearrange("(o p) t -> p o t", p=P)

            def xtc(ec, t0, t1):
                """View of xT columns t0:t1 (within one 512 chunk) at ec."""
                c = t0 // 512
                assert t1 <= (c + 1) * 512
                return xTs[c][:, ec, t0 - c * 512:t1 - c * 512]

            qt_t, kt_t, vp_t = {}, {}, {}

            def proj_ops(hp, by_chunk=False):
                pre = []
                qb, kb, vb = [], [], []
                st = {}
                # prologue weights ride the Activation hwdge queue so they
                # don't delay the x chunks on the SP queue
                dma_eng = nc.scalar if hp == 0 else nc.sync

                def wload():
                    st["w"] = w2.tile([P, EC, 2, P], bf16, tag="wdma",
                                      name=f"w{hp}")
                    for wi, w_dram in enumerate((wqt, wkt)):
                        dma_eng.dma_start(
                            st["w"][:, :, wi, :],
                            w_dram.rearrange("(o p) f -> p o f", p=P)[
                                :, :, hp * P:(hp + 1) * P])
                pre.append(wload)

                if hp % 2 == 0:
                    def vload():
                        st["wv"] = w2.tile([P, EC, 2 * P], bf16, tag="wdma_v",
                                           name=f"wv{hp}")
                        dma_eng.dma_start(
                            st["wv"][:],
                            wvt.rearrange("(o p) f -> p o f", p=P)[
                                :, :, hp * P:(hp + 2) * P])
                        vp_t[hp // 2] = vp_pool.tile(
                            [P, TC, 4, 65], bf16, tag="vp",
                            name=f"vp{hp // 2}")
                        nc.vector.tensor_copy(
                            vp_t[hp // 2][:, :, :, 64:65],
                            ones_col[:, None, None, :].to_broadcast(
                                [P, TC, 4, 1]))
                    pre.append(vload)

                # QT: two q-half tiles (separate tiles -> a query-half's
                # scores only depend on its own projection copies)
                for nq in range(SQ // 512):
                    blk = []

                    def qalloc(nq=nq):
                        if nq == 0:
                            qt_t[hp] = [
                                w2.tile([P, 512], bf16, tag=f"qt{j}",
                                        name=f"qt{hp}_{j}")
                                for j in range(2)]
                        st["pq"] = ps_p.tile([P, 512], f32, tag="PROJ",
                                             name=f"pq{hp}_{nq}")
                    blk.append(qalloc)
                    for ec in range(EC):
                        def qmm(ec=ec, nq=nq):
                            nc.tensor.matmul(
                                st["pq"][:], st["w"][:, ec, 0],
                                xtc(ec, nq * 512, (nq + 1) * 512),
                                start=(ec == 0), stop=(ec == EC - 1))
                        blk.append(qmm)

                    def qcopy(nq=nq):
                        nc.vector.tensor_copy(qt_t[hp][nq][:], st["pq"][:])
                    blk.append(qcopy)
                    qb.append(blk)

                # KT: four 512-chunks
                for nk in range(S // 512):
                    blk = []

                    def kalloc(nk=nk):
                        if nk == 0:
                            kt_t[hp] = [
                                w2.tile([P, 512], bf16, tag=f"kt{j}",
                                        name=f"kt{hp}_{j}")
                                for j in range(4)]
                        st["pk"] = ps_p.tile([P, 512], f32, tag="PROJ",
                                             name=f"pk{hp}_{nk}")
                    blk.append(kalloc)
                    for ec in range(EC):
                        def kmm(ec=ec, nk=nk):
                            nc.tensor.matmul(
                                st["pk"][:], st["w"][:, ec, 1],
                                xtc(ec, nk * 512, (nk + 1) * 512),
                                start=(ec == 0), stop=(ec == EC - 1))
                        blk.append(kmm)

                    def kcopy(nk=nk):
                        nc.vector.tensor_copy(kt_t[hp][nk][:], st["pk"][:])
                    blk.append(kcopy)
                    kb.append(blk)

                # V for the pair (hp, hp+1) on even hp: out free dim 256
                if hp % 2 == 0:
                    for tc_i in range(TC):
                        blk = []

                        def valloc(tc_i=tc_i):
                            st["pv"] = ps_p.tile([P, 512], f32, tag="PROJ",
                                                 name=f"pv{hp}_{tc_i}")
                        blk.append(valloc)
                        for ec in range(EC):
                            def vmm(ec=ec, tc_i=tc_i):
                                nc.tensor.matmul(
                                    st["pv"][:, :256],
                                    xtc(ec, tc_i * P, (tc_i + 1) * P),
                                    st["wv"][:, ec, :],
                                    start=(ec == 0), stop=(ec == EC - 1))
                            blk.append(vmm)

                        def vcopy(tc_i=tc_i):
                            nc.vector.tensor_copy(
                                vp_t[hp // 2][:, tc_i, :, 0:64],
                                st["pv"][:, :256].rearrange(
                                    "p (h d) -> p h d", h=4))
                        blk.append(vcopy)
                        vb.append(blk)

                ops = []
                if by_chunk:
                    # hp0 runs against the in-flight x DMAs: emit Q/K for
                    # each chunk as it lands, and use V blocks (which only
                    # need already-landed chunks) as filler so the PE never
                    # outpaces the DMA queue
                    order = [qb[0], kb[0], qb[1], kb[1],
                             vb[0], vb[1], vb[2], vb[3],
                             kb[2], vb[4], vb[5], vb[6], vb[7],
                             kb[3]] + vb[8:]
                    for blk in order:
                        ops += blk
                else:
                    # V blocks before the last two K chunks: the next
                    # block's first probs@V needs the final V copy, while
                    # K chunks 2-3 aren't read until its kc=8
                    for blk in qb + kb[:2] + vb + kb[2:]:
                        ops += blk
                return pre, ops

            # prologue: head-pair 0's weight DMAs (Activation queue), then
            # x DMAs, then head-pair 1's weight DMAs, then hp0's projections
            pre0, rest0 = proj_ops(0, by_chunk=True)
            pre_d, rest_d = {}, {}
            pre_d[1], rest_d[1] = proj_ops(1)
            for op in pre0:
                op()
            for i in range(XCH):
                nc.sync.dma_start(xTs[i][:], xt_view[:, :, i * 512:(i + 1) * 512])
            for op in pre_d[1]:
                op()
            for op in rest0:
                op()

            spill = {0: []}
            for hp in range(HP):
                qt, kt = qt_t[hp], kt_t[hp]
                vp = vp_t[hp // 2]
                ha, hb = 2 * (hp % 2), 2 * (hp % 2) + 1
                pace = {0: TC, 1: TC}
                if hp + 1 < HP:
                    # weight DMAs for head-pair hp+2 go out one block early
                    # (double-buffered w tiles), so hp+1's projections never
                    # wait on their weights mid-block
                    allops = []
                    if hp + 2 < HP:
                        pre_d[hp + 2], rest_d[hp + 2] = proj_ops(hp + 2)
                        allops += pre_d[hp + 2]
                    allops += rest_d[hp + 1]
                    if hp + 1 == HP - 1:
                        # head-pair 7's last two K chunks (its final 20 ops;
                        # only read from its kc=8 on) move into hp7's first
                        # query-half, which otherwise has no interleave work
                        spill[0] = allops[-20:]
                        allops = allops[:-20]
                    nxt = {0: allops[:len(allops) // 2],
                           1: allops[len(allops) // 2:]}
                else:
                    # last head-pair: K spill (front-loaded so the copies
                    # land before kc=8 needs them), then first-half
                    # out-projection in the second query-half's stream
                    nxt = {0: spill[0], 1: outproj_ops(0, ps_p)}
                    pace = {0: 7, 1: TC}

                for qh in range(2):
                    ops_q = nxt[qh]
                    n_emit = 0
                    po_a = ps_o.tile([65, 512], f32, tag="po")
                    po_b = ps_o.tile([65, 512], f32, tag="po")
                    qth = qt[qh]

                    def pv(kc, ut):
                        nc.tensor.matmul(
                            po_a[:], vp[:, kc, ha], ut[:, 0:512],
                            start=(kc == 0), stop=(kc == TC - 1))
                        nc.tensor.matmul(
                            po_b[:], vp[:, kc, hb], ut[:, 512:1024],
                            start=(kc == 0), stop=(kc == TC - 1))

                    prev = []
                    for kc in range(TC):
                        ktc = kt[kc // 4]
                        kcs = slice((kc % 4) * P, (kc % 4 + 1) * P)
                        sc = ps.tile([P, 1024], f32, tag="S")
                        nc.tensor.matmul(
                            sc[:, 0:512], ktc[0:64, kcs],
                            qth[0:64, :], start=True, stop=True)
                        nc.tensor.matmul(
                            sc[:, 512:1024], ktc[64:128, kcs],
                            qth[64:128, :], start=True, stop=True)
                        ut = ut_pool.tile([P, 1024], bf16, tag="ut")
                        nc.scalar.activation(
                            ut[:], sc[:], AF.Exp, scale=0.125)
                        # emit probs@V two kc behind the scores: the PE
                        # always has exp-independent work queued while ACT
                        # computes
                        if len(prev) == 3:
                            pv(*prev.pop(0))
                        prev.append((kc, ut))
                        # interleave pipelined work
                        target = min(len(ops_q),
                                     len(ops_q) * (kc + 1) // pace[qh])
                        while n_emit < target:
                            ops_q[n_emit]()
                            n_emit += 1
                    for pr in prev:
                        pv(*pr)
                    assert n_emit == len(ops_q)

                    # evacuate po immediately (frees the PSUM banks; a DVE
                    # op may read PSUM across partitions, so head b lands at
                    # partitions 64:128), then normalize: row 64 of each po
                    # is the softmax denominator. partition_broadcast only
                    # writes correctly from base 0: broadcast into a full
                    # tile, slice at read time.
                    po_s = w1.tile([P, 512], f32, tag="po_s")
                    nc.vector.tensor_copy(po_s[0:64, :], po_a[0:64, :])
                    rcp_a = w1.tile([1, 512], f32, tag="rcp_a")
                    nc.vector.reciprocal(rcp_a[:], po_a[64:65, :])
                    nc.vector.tensor_copy(po_s[64:128, :], po_b[0:64, :])
                    rcp_b = w1.tile([1, 512], f32, tag="rcp_b")
                    nc.vector.reciprocal(rcp_b[:], po_b[64:65, :])
                    brec_a = w1.tile([P, 512], f32, tag="brec_a")
                    nc.gpsimd.partition_broadcast(brec_a[:], rcp_a[:])
                    brec_b = w1.tile([P, 512], f32, tag="brec_b")
                    nc.gpsimd.partition_broadcast(brec_b[:], rcp_b[:])
                    nc.vector.tensor_mul(
                        onT[qh][hp][0:64, :], po_s[0:64, :], brec_a[0:64, :])
                    nc.vector.tensor_mul(
                        onT[qh][hp][64:128, :], po_s[64:128, :],
                        brec_b[64:128, :])

                # prefetch out-projection weights mid-attention
                if hp == 2:
                    prefetch_wo()

        # ---- output projection tail: second query-half ----
        with ExitStack() as dctx:
            ps_t = dctx.enter_context(
                tc.tile_pool(name="ps_t", bufs=3, space="PSUM"))
            for op in outproj_ops(1, ps_t):
                op()


def _build_kernel(reps=1):
    nc = bacc.Bacc("TRN2", target_bir_lowering=False, debug=False,
                   num_devices=N_CORES)
    xt_d = nc.dram_tensor("xt", [E, S], dt.bfloat16,
                          kind="ExternalInput").ap()
    wqt = nc.dram_tensor("wqt", [E, E], dt.bfloat16,
                         kind="ExternalInput").ap()
    wkt = nc.dram_tensor("wkt", [E, E], dt.bfloat16,
                         kind="ExternalInput").ap()
    wvt = nc.dram_tensor("wvt", [E, E], dt.bfloat16,
                         kind="ExternalInput").ap()
    wot = nc.dram_tensor("wot", [E, E], dt.bfloat16,
                         kind="ExternalInput").ap()
    bo = nc.dram_tensor("bo", [1, E], dt.float32, kind="ExternalInput").ap()
    y = nc.dram_tensor("y", [SQ, E], dt.float32, kind="ExternalOutput").ap()

    with tile.TileContext(nc) as tc:
        for _ in range(reps):
            _emit(nc, tc, xt_d, wqt, wkt, wvt, wot, bo, y)
    nc.compile()
    return nc


_NC_CACHE = None


def _bf16(a):
    import ml_dtypes
    return np.ascontiguousarray(np.asarray(a, np.float32).astype(
        ml_dtypes.bfloat16))


def make_in_maps(x, Wq, Wk, Wv, Wo, bo):
    x = np.asarray(x, np.float32)
    wqt = _bf16(np.asarray(Wq, np.float32).T)
    wkt = _bf16(np.asarray(Wk, np.float32).T)
    wvt = _bf16(np.asarray(Wv, np.float32).T)
    wot = _bf16(np.asarray(Wo, np.float32).T)
    bo_ = np.ascontiguousarray(np.asarray(bo, np.float32).reshape(1, E))

    in_maps = []
    for c in range(N_CORES):
        b, half = c // 2, c % 2
        # rotate so this core's query block is rows 0:SQ (keys are a
        # permutation of the sequence -- attention is invariant to key order)
        xt_rot = _bf16(np.roll(x[b], -half * SQ, axis=0).T)
        in_maps.append({"xt": xt_rot, "wqt": wqt, "wkt": wkt, "wvt": wvt,
                        "wot": wot, "bo": bo_})
    return in_maps


def get_nc(reps=1):
    global _NC_CACHE
    if _NC_CACHE is None:
        _NC_CACHE = {}
    if reps not in _NC_CACHE:
        _NC_CACHE[reps] = _build_kernel(reps)
    return _NC_CACHE[reps]


def kernel(x, Wq, Wk, Wv, Wo, bo):
    nc = get_nc()
    in_maps = make_in_maps(x, Wq, Wk, Wv, Wo, bo)
    res = run_bass_kernel_spmd(nc, in_maps, core_ids=list(range(N_CORES)))
    out = np.empty((B, S, E), np.float32)
    for c in range(N_CORES):
        b, half = c // 2, c % 2
        out[b, half * SQ:(half + 1) * SQ, :] = res.results[c]["y"]
    return out


# revision 36
# speedup vs baseline: 1.0773x; 1.0022x over previous
"""Multi-head attention (B=4, S=2048, E=1024, H=16, D=64) on 8 TRN2 cores.

Sharding: core c handles batch b = c//2, query half = c%2 (1024 queries).
Each core computes K/V over its batch's full sequence (duplicated between the
two half-cores of a batch -- cheaper at these sizes than any collective),
attention for all 16 heads over its 1024 queries, and the output projection
for its output chunk. Outputs are disjoint -> host gather is concatenation.

The host rotates each core's sequence so its query block is always rows
0:1024 (attention is permutation-invariant over keys), pre-transposes the
weights and activations (pure layout prep) so the e-contraction projections
have e on partitions, and converts everything to bf16 (rel-err budget 2e-2;
bf16 lands ~3e-3).

bf16 operands keep the PE at full rate, enable Fast Weight Load, halve DMA
bytes, and remove all dtype-staging copies: DMAs land directly in the SBUF
tiles the matmuls read. PSUM accumulation stays fp32.

Scores matmuls use contraction 64 (head dim) at base partitions 0/64, which
bass auto-lowers to 64x128 row tiles (0,0)/(64,0) -- on hardware the two
head matmuls run concurrently in the two half-arrays.

Schedule: the first head-pair's weight DMAs go out before the x DMAs (so the
PE starts ~5us in, not ~16us); head-pair hp+1's projection instructions are
interleaved into hp's attention stream; the out-projection weights prefetch
mid-attention; the first query-half's out-projection interleaves into the
last head-pair's attention; softmax state is copied out of PSUM immediately
so the po banks recycle without waiting on the normalize chain.
"""

from contextlib import ExitStack

import numpy as np

import concourse.tile as tile
from concourse import bacc, mybir
from concourse.bass_utils import run_bass_kernel_spmd

dt = mybir.dt
AF = mybir.ActivationFunctionType

B, S, E, H, D = 4, 2048, 1024, 16, 64
N_CORES = 8
SQ = 1024          # queries per core
P = 128
EC = E // P        # 8 e-chunks
TC = S // P        # 16 t-chunks (keys)
QC = SQ // P       # 8 query chunks
HP = H // 2        # 8 head-pairs
XCH = 4            # xT token chunks (512 tokens each)


def _emit(nc, tc, xt_d, wqt, wkt, wvt, wot, bo, y):
    f32, bf16 = dt.float32, dt.bfloat16

    with ExitStack() as ctx:
        const = ctx.enter_context(tc.tile_pool(name="const", bufs=1))
        ps_p = ctx.enter_context(
            tc.tile_pool(name="ps_p", bufs=2, space="PSUM"))
        on_pool = ctx.enter_context(tc.tile_pool(name="on", bufs=1))
        wo_pool = ctx.enter_context(tc.tile_pool(name="wo", bufs=2))
        yp = ctx.enter_context(tc.tile_pool(name="yp", bufs=4))

        ones_col = const.tile([P, 1], bf16)
        nc.vector.memset(ones_col[:], 1.0)

        # warm the PE (p-state / HAM) with dependency-free matmuls while the
        # first DMAs are in flight
        wu = const.tile([P, 512], bf16)
        nc.vector.memset(wu[:], 0.0)

        # attention output, [e, q] layout: partition j of tile (qh, hp) is
        # e-row hp*128+j (head 2hp on partitions 0:64, 2hp+1 on 64:128).
        # One tile per (query-half, head-pair) so out-projection reads only
        # depend on the normalizes that actually produced them.
        onT = [[on_pool.tile([P, 512], bf16, tag=f"on{qh}_{hp}",
                             name=f"onT{qh}_{hp}")
                for hp in range(HP)]
               for qh in range(2)]
        wo_rs = []
        bo_rep = [None]

        def prefetch_wo():
            bo_one = wo_pool.tile([1, E], f32, tag="bo1")
            nc.sync.dma_start(bo_one[:], bo[:])
            bo_rep[0] = wo_pool.tile([P, E], f32, tag="bor", name="bo_rep")
            nc.gpsimd.partition_broadcast(bo_rep[0][:], bo_one[:])
            wot_view = wot.rearrange("(o p) f -> p o f", p=P)
            for nf in range(E // 512):
                wo_sb = wo_pool.tile([P, EC, 512], bf16, tag="wosb")
                nc.sync.dma_start(
                    wo_sb[:], wot_view[:, :, nf * 512:(nf + 1) * 512])
                wo_rs.append(wo_sb)

        def outproj_ops(qh, pool):
            """Out-projection for query-half qh: reads onT[qh] tiles."""
            ops = []
            st = {}
            for qc in range(4 * qh, 4 * qh + 4):
                for nf in range(E // 512):
                    def palloc(qc=qc, nf=nf):
                        st["py"] = pool.tile([P, 512], f32, tag="PROJ",
                                             name=f"py{qc}_{nf}")
                    ops.append(palloc)
                    for hp in range(HP):
                        def pmm(hp=hp, qc=qc, nf=nf):
                            nc.tensor.matmul(
                                st["py"][:],
                                onT[qh][hp][:, (qc % 4) * P:(qc % 4 + 1) * P],
                                wo_rs[nf][:, hp, :],
                                start=(hp == 0), stop=(hp == HP - 1))
                        ops.append(pmm)

                    def pout(qc=qc, nf=nf):
                        y_sb = yp.tile([P, 512], bf16, tag="ysb")
                        nc.vector.tensor_add(
                            y_sb[:], st["py"][:],
                            bo_rep[0][:, nf * 512:(nf + 1) * 512])
                        # alternate hwdge queues so the final stores drain
                        # in parallel
                        eng = nc.sync if (qc + nf) % 2 == 0 else nc.scalar
                        eng.dma_start(
                            y[qc * P:(qc + 1) * P, nf * 512:(nf + 1) * 512],
                            y_sb[:])
                    ops.append(pout)
            return ops

        with ExitStack() as actx:
            ps = actx.enter_context(
                tc.tile_pool(name="ps", bufs=2, space="PSUM"))
            wu_p = ps.tile([P, 512], f32, tag="S", name="wu_p")
            for _ in range(8):
                nc.tensor.matmul(wu_p[:], wu[:, 0:P], wu[:],
                                 start=True, stop=True)
            ps_o = actx.enter_context(
                tc.tile_pool(name="ps_o", bufs=2, space="PSUM"))
            xt_pool = actx.enter_context(tc.tile_pool(name="xt", bufs=1))
            w1 = actx.enter_context(tc.tile_pool(name="w1", bufs=1))
            w2 = actx.enter_context(tc.tile_pool(name="w2", bufs=2))
            vp_pool = actx.enter_context(tc.tile_pool(name="vp", bufs=2))
            ut_pool = actx.enter_context(tc.tile_pool(name="ut", bufs=6))

            # xT in SBUF (bf16), 4 token-chunk tiles; queries are chunks 0:2
            xTs = [xt_pool.tile([P, EC, 512], bf16, tag=f"xt{i}",
                            name=f"xT{i}")
                   for i in range(XCH)]
            xt_view = xt_d.rearrange("(o p) t -> p o t", p=P)

            def xtc(ec, t0, t1):
                """View of xT columns t0:t1 (within one 512 chunk) at ec."""
                c = t0 // 512
                assert t1 <= (c + 1) * 512
                return xTs[c][:, ec, t0 - c * 512:t1 - c * 512]

            qt_t, kt_t, vp_t = {}, {}, {}

            def proj_ops(hp, by_chunk=False):
                pre = []
                qb, kb, vb = [], [], []
                st = {}
                # prologue weights ride the Activation hwdge queue so they
                # don't delay the x chunks on the SP queue
                dma_eng = nc.scalar if hp == 0 else nc.sync

                def wload():
                    st["w"] = w2.tile([P, EC, 2, P], bf16, tag="wdma",
                                      name=f"w{hp}")
                    wq_v = wqt.rearrange("(o p) f -> p o f", p=P)
                    wk_v = wkt.rearrange("(o p) f -> p o f", p=P)
                    fs = slice(hp * P, (hp + 1) * P)
                    if hp == 0:
                        # split so Q's first e-chunks can start ~3us earlier
                        dma_eng.dma_start(st["w"][:, 0:2, 0, :],
                                          wq_v[:, 0:2, fs])
                        dma_eng.dma_start(st["w"][:, 2:EC, 0, :],
                                          wq_v[:, 2:EC, fs])
                    else:
                        dma_eng.dma_start(st["w"][:, :, 0, :], wq_v[:, :, fs])
                    dma_eng.dma_start(st["w"][:, :, 1, :], wk_v[:, :, fs])
                pre.append(wload)

                if hp % 2 == 0:
                    def vload():
                        st["wv"] = w2.tile([P, EC, 2 * P], bf16, tag="wdma_v",
                                           name=f"wv{hp}")
                        dma_eng.dma_start(
                            st["wv"][:],
                            wvt.rearrange("(o p) f -> p o f", p=P)[
                                :, :, hp * P:(hp + 2) * P])
                        vp_t[hp // 2] = vp_pool.tile(
                            [P, TC, 4, 65], bf16, tag="vp",
                            name=f"vp{hp // 2}")
                        nc.vector.tensor_copy(
                            vp_t[hp // 2][:, :, :, 64:65],
                            ones_col[:, None, None, :].to_broadcast(
                                [P, TC, 4, 1]))
                    pre.append(vload)

                # QT: two q-half tiles (separate tiles -> a query-half's
                # scores only depend on its own projection copies)
                for nq in range(SQ // 512):
                    blk = []

                    def qalloc(nq=nq):
                        if nq == 0:
                            qt_t[hp] = [
                                w2.tile([P, 512], bf16, tag=f"qt{j}",
                                        name=f"qt{hp}_{j}")
                                for j in range(2)]
                        st["pq"] = ps_p.tile([P, 512], f32, tag="PROJ",
                                             name=f"pq{hp}_{nq}")
                    blk.append(qalloc)
                    for ec in range(EC):
                        def qmm(ec=ec, nq=nq):
                            nc.tensor.matmul(
                                st["pq"][:], st["w"][:, ec, 0],
                                xtc(ec, nq * 512, (nq + 1) * 512),
                                start=(ec == 0), stop=(ec == EC - 1))
                        blk.append(qmm)

                    def qcopy(nq=nq):
                        nc.vector.tensor_copy(qt_t[hp][nq][:], st["pq"][:])
                    blk.append(qcopy)
                    qb.append(blk)

                # KT: four 512-chunks
                for nk in range(S // 512):
                    blk = []

                    def kalloc(nk=nk):
                        if nk == 0:
                            kt_t[hp] = [
                                w2.tile([P, 512], bf16, tag=f"kt{j}",
                                        name=f"kt{hp}_{j}")
                                for j in range(4)]
                        st["pk"] = ps_p.tile([P, 512], f32, tag="PROJ",
                                             name=f"pk{hp}_{nk}")
                    blk.append(kalloc)
                    for ec in range(EC):
                        def kmm(ec=ec, nk=nk):
                            nc.tensor.matmul(
                                st["pk"][:], st["w"][:, ec, 1],
                                xtc(ec, nk * 512, (nk + 1) * 512),
                                start=(ec == 0), stop=(ec == EC - 1))
                        blk.append(kmm)

                    def kcopy(nk=nk):
                        nc.vector.tensor_copy(kt_t[hp][nk][:], st["pk"][:])
                    blk.append(kcopy)
                    kb.append(blk)

                # V for the pair (hp, hp+1) on even hp: out free dim 256
                if hp % 2 == 0:
                    for tc_i in range(TC):
                        blk = []

                        def valloc(tc_i=tc_i):
                            st["pv"] = ps_p.tile([P, 512], f32, tag="PROJ",
                                                 name=f"pv{hp}_{tc_i}")
                        blk.append(valloc)
                        for ec in range(EC):
                            def vmm(ec=ec, tc_i=tc_i):
                                nc.tensor.matmul(
                                    st["pv"][:, :256],
                                    xtc(ec, tc_i * P, (tc_i + 1) * P),
                                    st["wv"][:, ec, :],
                                    start=(ec == 0), stop=(ec == EC - 1))
                            blk.append(vmm)

                        def vcopy(tc_i=tc_i):
                            nc.vector.tensor_copy(
                                vp_t[hp // 2][:, tc_i, :, 0:64],
                                st["pv"][:, :256].rearrange(
                                    "p (h d) -> p h d", h=4))
                        blk.append(vcopy)
                        vb.append(blk)

                ops = []
                if by_chunk:
                    # hp0 runs against the in-flight x DMAs: emit Q/K for
                    # each chunk as it lands, and use V blocks (which only
                    # need already-landed chunks) as filler so the PE never
                    # outpaces the DMA queue
                    order = [qb[0], kb[0], qb[1], kb[1],
                             vb[0], vb[1], vb[2], vb[3],
                             kb[2], vb[4], vb[5], vb[6], vb[7],
                             kb[3]] + vb[8:]
                    for blk in order:
                        ops += blk
                else:
                    # V blocks before the last two K chunks: the next
                    # block's first probs@V needs the final V copy, while
                    # K chunks 2-3 aren't read until its kc=8
                    for blk in qb + kb[:2] + vb + kb[2:]:
                        ops += blk
                return pre, ops

            # prologue: head-pair 0's weight DMAs (Activation queue), then
            # x DMAs, then head-pair 1's weight DMAs, then hp0's projections
            pre0, rest0 = proj_ops(0, by_chunk=True)
            pre_d, rest_d = {}, {}
            pre_d[1], rest_d[1] = proj_ops(1)
            for op in pre0:
                op()
            for j in range(4):
                nc.sync.dma_start(xTs[0][:, 2 * j:2 * j + 2, :],
                                  xt_view[:, 2 * j:2 * j + 2, 0:512])
            for i in range(1, XCH):
                nc.sync.dma_start(xTs[i][:], xt_view[:, :, i * 512:(i + 1) * 512])
            for op in pre_d[1]:
                op()
            for op in rest0:
                op()

            spill = {0: []}
            for hp in range(HP):
                qt, kt = qt_t[hp], kt_t[hp]
                vp = vp_t[hp // 2]
                ha, hb = 2 * (hp % 2), 2 * (hp % 2) + 1
                pace = {0: TC, 1: TC}
                if hp + 1 < HP:
                    # weight DMAs for head-pair hp+2 go out one block early
                    # (double-buffered w tiles), so hp+1's projections never
                    # wait on their weights mid-block
                    allops = []
                    if hp + 2 < HP:
                        pre_d[hp + 2], rest_d[hp + 2] = proj_ops(hp + 2)
                        allops += pre_d[hp + 2]
                    allops += rest_d[hp + 1]
                    if hp + 1 == HP - 1:
                        # head-pair 7's last two K chunks (its final 20 ops;
                        # only read from its kc=8 on) move into hp7's first
                        # query-half, which otherwise has no interleave work
                        spill[0] = allops[-20:]
                        allops = allops[:-20]
                    nxt = {0: allops[:len(allops) // 2],
                           1: allops[len(allops) // 2:]}
                else:
                    # last head-pair: K spill (front-loaded so the copies
                    # land before kc=8 needs them), then first-half
                    # out-projection in the second query-half's stream
                    nxt = {0: spill[0], 1: outproj_ops(0, ps_p)}
                    pace = {0: 7, 1: TC}

                for qh in range(2):
                    ops_q = nxt[qh]
                    n_emit = 0
                    po_a = ps_o.tile([65, 512], f32, tag="po")
                    po_b = ps_o.tile([65, 512], f32, tag="po")
                    qth = qt[qh]

                    def pv(kc, ut):
                        nc.tensor.matmul(
                            po_a[:], vp[:, kc, ha], ut[:, 0:512],
                            start=(kc == 0), stop=(kc == TC - 1))
                        nc.tensor.matmul(
                            po_b[:], vp[:, kc, hb], ut[:, 512:1024],
                            start=(kc == 0), stop=(kc == TC - 1))

                    prev = []
                    for kc in range(TC):
                        ktc = kt[kc // 4]
                        kcs = slice((kc % 4) * P, (kc % 4 + 1) * P)
                        sc = ps.tile([P, 1024], f32, tag="S")
                        nc.tensor.matmul(
                            sc[:, 0:512], ktc[0:64, kcs],
                            qth[0:64, :], start=True, stop=True)
                        nc.tensor.matmul(
                            sc[:, 512:1024], ktc[64:128, kcs],
                            qth[64:128, :], start=True, stop=True)
                        ut = ut_pool.tile([P, 1024], bf16, tag="ut")
                        nc.scalar.activation(
                            ut[:], sc[:], AF.Exp, scale=0.125)
                        # emit probs@V two kc behind the scores: the PE
                        # always has exp-independent work queued while ACT
                        # computes
                        if len(prev) == 3:
                            pv(*prev.pop(0))
                        prev.append((kc, ut))
                        # interleave pipelined work
                        target = min(len(ops_q),
                                     len(ops_q) * (kc + 1) // pace[qh])
                        while n_emit < target:
                            ops_q[n_emit]()
                            n_emit += 1
                    for pr in prev:
                        pv(*pr)
                    assert n_emit == len(ops_q)

                    # evacuate po immediately (frees the PSUM banks; a DVE
                    # op may read PSUM across partitions, so head b lands at
                    # partitions 64:128), then normalize: row 64 of each po
                    # is the softmax denominator. partition_broadcast only
                    # writes correctly from base 0: broadcast into a full
                    # tile, slice at read time.
                    po_s = w1.tile([P, 512], f32, tag="po_s")
                    nc.vector.tensor_copy(po_s[0:64, :], po_a[0:64, :])
                    rcp_a = w1.tile([1, 512], f32, tag="rcp_a")
                    nc.vector.reciprocal(rcp_a[:], po_a[64:65, :])
                    nc.vector.tensor_copy(po_s[64:128, :], po_b[0:64, :])
                    rcp_b = w1.tile([1, 512], f32, tag="rcp_b")
                    nc.vector.reciprocal(rcp_b[:], po_b[64:65, :])
                    brec_a = w1.tile([P, 512], f32, tag="brec_a")
                    nc.gpsimd.partition_broadcast(brec_a[:], rcp_a[:])
                    brec_b = w1.tile([P, 512], f32, tag="brec_b")
                    nc.gpsimd.partition_broadcast(brec_b[:], rcp_b[:])
                    nc.vector.tensor_mul(
                        onT[qh][hp][0:64, :], po_s[0:64, :], brec_a[0:64, :])
                    nc.vector.tensor_mul(
                        onT[qh][hp][64:128, :], po_s[64:128, :],
                        brec_b[64:128, :])

                # prefetch out-projection weights mid-attention
                if hp == 2:
                    prefetch_wo()

        # ---- output projection tail: second query-half ----
        with ExitStack() as dctx:
            ps_t = dctx.enter_context(
                tc.tile_pool(name="ps_t", bufs=3, space="PSUM"))
            for op in outproj_ops(1, ps_t):
                op()


def _build_kernel(reps=1):
    nc = bacc.Bacc("TRN2", target_bir_lowering=False, debug=False,
                   num_devices=N_CORES)
    xt_d = nc.dram_tensor("xt", [E, S], dt.bfloat16,
                          kind="ExternalInput").ap()
    wqt = nc.dram_tensor("wqt", [E, E], dt.bfloat16,
                         kind="ExternalInput").ap()
    wkt = nc.dram_tensor("wkt", [E, E], dt.bfloat16,
                         kind="ExternalInput").ap()
    wvt = nc.dram_tensor("wvt", [E, E], dt.bfloat16,
                         kind="ExternalInput").ap()
    wot = nc.dram_tensor("wot", [E, E], dt.bfloat16,
                         kind="ExternalInput").ap()
    bo = nc.dram_tensor("bo", [1, E], dt.float32, kind="ExternalInput").ap()
    y = nc.dram_tensor("y", [SQ, E], dt.bfloat16,
                       kind="ExternalOutput").ap()

    with tile.TileContext(nc) as tc:
        for _ in range(reps):
            _emit(nc, tc, xt_d, wqt, wkt, wvt, wot, bo, y)
    nc.compile()
    return nc


_NC_CACHE = None


def _bf16(a):
    import ml_dtypes
    return np.ascontiguousarray(np.asarray(a, np.float32).astype(
        ml_dtypes.bfloat16))


def make_in_maps(x, Wq, Wk, Wv, Wo, bo):
    x = np.asarray(x, np.float32)
    wqt = _bf16(np.asarray(Wq, np.float32).T)
    wkt = _bf16(np.asarray(Wk, np.float32).T)
    wvt = _bf16(np.asarray(Wv, np.float32).T)
    wot = _bf16(np.asarray(Wo, np.float32).T)
    bo_ = np.ascontiguousarray(np.asarray(bo, np.float32).reshape(1, E))

    in_maps = []
    for c in range(N_CORES):
        b, half = c // 2, c % 2
        # rotate so this core's query block is rows 0:SQ (keys are a
        # permutation of the sequence -- attention is invariant to key order)
        xt_rot = _bf16(np.roll(x[b], -half * SQ, axis=0).T)
        in_maps.append({"xt": xt_rot, "wqt": wqt, "wkt": wkt, "wvt": wvt,
                        "wot": wot, "bo": bo_})
    return in_maps


def get_nc(reps=1):
    global _NC_CACHE
    if _NC_CACHE is None:
        _NC_CACHE = {}
    if reps not in _NC_CACHE:
        _NC_CACHE[reps] = _build_kernel(reps)
    return _NC_CACHE[reps]


def kernel(x, Wq, Wk, Wv, Wo, bo):
    nc = get_nc()
    in_maps = make_in_maps(x, Wq, Wk, Wv, Wo, bo)
    res = run_bass_kernel_spmd(nc, in_maps, core_ids=list(range(N_CORES)))
    out = np.empty((B, S, E), np.float32)
    for c in range(N_CORES):
        b, half = c // 2, c % 2
        out[b, half * SQ:(half + 1) * SQ, :] = \
            res.results[c]["y"].astype(np.float32)
    return out
